# revision 1
# baseline (speedup 1.0000x reference)
"""Trainium2 Bass kernel for a batch-of-trees BinaryTreeLSTM.

Contract: kernel(**inputs) takes the FULL inputs (B=128 trees, 1023-node
complete binary tree, dim 300) and returns the FULL output (root_c, root_h),
each [128, 300] float32.

Strategy
--------
- Data-parallel over trees: 16 trees per NeuronCore x 8 cores, no collectives.
- The node scan is really 10 levels (512 leaves -> 256 -> ... -> 1 root).
  Internal nodes have zero input (leaf_mask), so their input-gate term is just
  the bias; the big `inputs` tensor only matters for the 512 leaf rows.
- Everything is computed feature-on-partitions (transposed): states are
  [300, cols] with col = node*16 + tree.  Host pre-transposes the leaf inputs.
- Per level: one GEMM [cols, 600] @ [600, 1500] (i,o,u,fL,fR gates fused,
  weights stationary as lhsT), ACT applies sigmoid/tanh + bias from PSUM,
  DVE/GPSIMD do the elementwise recurrence.
- Matmuls use float32r (FP22 truncation) -> 1 cycle/row at N>=256, full
  bf16-speed with ~2^-12 relative precision.
- States are stored DEINTERLEAVED ([even nodes | odd nodes]) so the next
  level's children gather is a dense slice; the two 44-partition tail chunks
  of the contraction (features 256:300 of left/right child h) are packed into
  one K-chunk tile at partitions 0:44 / 64:108, giving 5 K-chunks instead of 6.
- Levels 1..3 stream through DRAM (too big for SBUF); levels 4..9 stay
  SBUF-resident.
"""

import os
import sys

for _p in ("/opt/trn_rl_repo",):
    if os.path.isdir(_p) and _p not in sys.path:
        sys.path.insert(0, _p)

import numpy as np
from contextlib import ExitStack

import concourse.bass as bass
import concourse.tile as tile
from concourse import bacc, mybir
from concourse.bass_utils import run_bass_kernel_spmd

# ---------------------------------------------------------------- constants
N_CORES = 8
B = 128
B_LOC = B // N_CORES          # 16 trees per core
N_LEAVES = 512
MEM = 300
XCOLS = N_LEAVES * B_LOC      # 8192 leaf columns per core
FCH = [(0, 128), (128, 256), (256, 300)]   # feature chunks
NCH = 3
R_LVL = [4096, 2048, 1024, 512, 256, 128, 64, 32, 16]  # cols at levels 1..9
N_DRAM_LVLS = 3               # levels 1..3 stream through DRAM
LB = 1024                     # leaf-block columns (64 leaves)
PB_MAX = 512                  # parent-block columns (recurrent levels)

DT = mybir.dt.float32
DTR = mybir.dt.float32r
AF = mybir.ActivationFunctionType
GATE_FUNC = [AF.Sigmoid, AF.Sigmoid, AF.Tanh, AF.Sigmoid, AF.Sigmoid]  # i,o,u,fL,fR


# ---------------------------------------------------------------- host packing
def _pack_weights(Wfioux, b_fioux, Wiouh, Wfh):
    f4 = np.float32
    Wiou = np.asarray(Wfioux[:, 300:1200], f4)            # [300, 900]
    wleaf = np.zeros((NCH, 128, 900), f4)
    for j, (a, b) in enumerate(FCH):
        wleaf[j, : b - a] = Wiou[a:b]
    Wcat = np.concatenate(
        [Wiouh[:, 0:300], Wiouh[:, 300:600], Wiouh[:, 600:900],
         Wfh[:, 0:300], Wfh[:, 300:600]], axis=1).astype(f4)  # [600, 1500]
    wrec = np.zeros((5, 128, 1500), f4)
    wrec[0, :128] = Wcat[0:128]
    wrec[1, :128] = Wcat[128:256]
    wrec[2, 0:44] = Wcat[256:300]        # left-child feat 256:300
    wrec[2, 64:108] = Wcat[556:600]      # right-child feat 256:300
    wrec[3, :128] = Wcat[300:428]
    wrec[4, :128] = Wcat[428:556]
    bf = b_fioux[0:300]
    bias_cat = np.concatenate(
        [b_fioux[300:600], b_fioux[600:900], b_fioux[900:1200], bf, bf]).astype(f4)
    # bias baked into the recurrent GEMM: ones-row at partition 44 of the
    # packed chunk-2 rhs multiplies this weight row
    wrec[2, 44] = bias_cat
    bias128 = np.zeros((17, 128), f4)
    for g in range(5):
        for j, (a, b) in enumerate(FCH):
            bias128[g * 3 + j, : b - a] = bias_cat[g * 300 + a: g * 300 + b]
    # packed-pair bias columns for the col-tiled 44-wide chunks:
    # col 15: i2 at parts 0:44, o2 at parts 64:108; col 16: fL2 / fR2
    bias128[15, 0:44] = bias_cat[256:300]
    bias128[15, 64:108] = bias_cat[556:600]
    bias128[16, 0:44] = bias_cat[1156:1200]
    bias128[16, 64:108] = bias_cat[1456:1500]
    biasT = np.ascontiguousarray(bias128.T)               # [128, 17]
    return wleaf, wrec, biasT


def _check_topology(left_idx, right_idx, leaf_mask):
    li = np.asarray(left_idx); ri = np.asarray(right_idx)
    prev = np.arange(N_LEAVES); nid = N_LEAVES
    ok = bool((np.asarray(leaf_mask)[:N_LEAVES] == 1).all())
    ok &= bool((np.asarray(leaf_mask)[N_LEAVES:] == 0).all())
    while len(prev) > 1:
        cur = []
        for k in range(0, len(prev), 2):
            ok &= bool(li[nid] == prev[k]) and bool(ri[nid] == prev[k + 1])
            cur.append(nid); nid += 1
        prev = np.asarray(cur)
    return ok


def _consts():
    c = np.zeros((130, LB), np.float32)
    c[44] = 1.0
    return c


# ---------------------------------------------------------------- bass program
def _even_odd(ap, half, b=B_LOC):
    """Split a [p, 2*half] block-ordered AP into (even-node cols, odd-node cols),
    each viewed as [p, half//b, b]."""
    r = ap.rearrange("p (m two b) -> p m two b", two=2, b=b)
    return r[:, :, 0, :], r[:, :, 1, :]


def build_program():
    """Builds the full Bass/Tile program. Returns (nc, input_names, output_name)."""
    nc = bacc.Bacc("TRN2", target_bir_lowering=False, debug=False)

    xt_d = nc.dram_tensor("xt", [MEM, XCOLS], DTR, kind="ExternalInput").ap()
    wleaf_d = nc.dram_tensor("wleaf", [NCH, 128, 900], DTR, kind="ExternalInput").ap()
    wrec_d = nc.dram_tensor("wrec", [5, 128, 1500], DTR, kind="ExternalInput").ap()
    bias_d = nc.dram_tensor("biasT", [128, 17], DT, kind="ExternalInput").ap()
    zeros_d = nc.dram_tensor("consts", [130, LB], DTR, kind="ExternalInput").ap()
    out_d = nc.dram_tensor("out", [2, MEM, B_LOC], DT, kind="ExternalOutput").ap()

    # DRAM state for levels 1..3 (deinterleaved col order).  Level 1 is split
    # into 4 column-range tensors so level-2 staging only depends on the
    # phase-A blocks that actually produced its columns.
    Hd = {}; Cd = {}
    Hd[1] = [nc.dram_tensor(f"Hs1_{k}", [MEM, 1024], DTR).ap() for k in range(4)]
    Cd[1] = [nc.dram_tensor(f"Cs1_{k}", [MEM, 1024], DT).ap() for k in range(4)]
    for lvl in range(2, N_DRAM_LVLS + 1):
        R = R_LVL[lvl - 1]
        Hd[lvl] = nc.dram_tensor(f"Hs{lvl}", [MEM, R], DTR).ap()
        Cd[lvl] = nc.dram_tensor(f"Cs{lvl}", [MEM, R], DT).ap()

    with ExitStack() as ctx:
        tc = ctx.enter_context(tile.TileContext(nc))
        _build_kernel(ctx, tc, xt_d, wleaf_d, wrec_d, bias_d, zeros_d, Hd, Cd, out_d)

    nc.compile()
    return nc


def _build_kernel(ctx, tc, xt_d, wleaf_d, wrec_d, bias_d, zeros_d, Hd, Cd, out_d):
    nc = tc.nc

    wpool = ctx.enter_context(tc.tile_pool(name="wpool", bufs=1))
    state_pool = ctx.enter_context(tc.tile_pool(name="state", bufs=1))
    tmp_pool = ctx.enter_context(tc.tile_pool(name="tmps", bufs=2))
    out_pool = ctx.enter_context(tc.tile_pool(name="outs", bufs=1))

    # ---- weights / bias resident in SBUF
    wrec_sb = []
    for k in range(5):
        t = wpool.tile([128, 1500], DTR, name=f"wrec{k}")
        nc.sync.dma_start(t[:], wrec_d[k])
        wrec_sb.append(t)
    wleaf_sb = []
    for k in range(NCH):
        t = wpool.tile([128, 900], DTR, name=f"wleaf{k}")
        nc.sync.dma_start(t[:], wleaf_d[k])
        wleaf_sb.append(t)
    bias_sb = wpool.tile([128, 17], DT, name="bias")
    nc.sync.dma_start(bias_sb[:], bias_d[:])

    # ---- persistent SBUF state for levels 4..9
    # H: t0,t1 [128, R] ([left|right]), t2p [128, R//2] (parts 0:44 left feat2,
    #    64:108 right feat2).  C: c0,c1 [128, R], c2 [44, R].
    Hsb = {}; Csb = {}
    for lvl in range(N_DRAM_LVLS + 1, 9):
        R = R_LVL[lvl - 1]
        h0 = state_pool.tile([128, R], DTR, name=f"H{lvl}_0")
        h1 = state_pool.tile([128, R], DTR, name=f"H{lvl}_1")
        h2 = state_pool.tile([128, R // 2], DTR, name=f"H{lvl}_2p")
        nc.sync.dma_start(h2[44:64, :], zeros_d[44:64, : R // 2])
        nc.sync.dma_start(h2[108:128, :], zeros_d[46:66, : R // 2])
        c0 = state_pool.tile([128, R], DT, name=f"C{lvl}_0")
        c1 = state_pool.tile([128, R], DT, name=f"C{lvl}_1")
        c2 = state_pool.tile([44, R], DT, name=f"C{lvl}_2")
        Hsb[lvl] = (h0, h1, h2)
        Csb[lvl] = (c0, c1, c2)

    def gemm_gates(rhs_k, PBn, pool_ps=None, n_free=512):
        """Gate matmuls + per-gate ACT (bias via ACT operand) for level 1.
        j0/j1: full 128-wide chunks; j2: the five 44-wide chunks run as
        col-tiled concurrent pairs (i2|o2), (fL2|fR2), plus u2."""
        gates = [[None] * NCH for _ in range(5)]

        def mms(ps, g, a, b, base, tile_pos, off=0):
            cw = b - a
            for n0 in range(0, PBn, n_free):
                n1 = min(n0 + n_free, PBn)
                for kc in range(5):
                    nc.tensor.matmul(
                        ps[base:base + cw, off + n0: off + n1],
                        wrec_sb[kc][:, 300 * g + a: 300 * g + b],
                        rhs_k[kc][:, n0:n1],
                        start=(kc == 0), stop=(kc == 4),
                        tile_position=tile_pos)

        for j, (a, b) in enumerate(FCH[:2]):
            for g in range(5):
                cw = b - a
                ps = pool_ps.tile([128, PB_MAX], mybir.dt.float32, tag="ps", name="ps")
                mms(ps, g, a, b, 0, None)
                gt = gate_pool.tile([cw, PBn], DT, tag=f"gate{g}", name=f"gate{g}_{j}")
                m = g * 3 + j
                nc.scalar.activation(gt[:], ps[:cw, :PBn], GATE_FUNC[g],
                                     bias=bias_sb[:cw, m:m + 1])
                gates[g][j] = gt
        # j2 pairs
        a, b = FCH[2]
        ps_io = pool_ps.tile([128, PB_MAX], mybir.dt.float32, tag="ps", name="ps_io2")
        ps_i2 = pool_ps.tile([128, PB_MAX], mybir.dt.float32, tag="ps", name="ps_i2")
        mms(ps_i2, 0, a, b, 0, None)
        g_i2 = gate_pool.tile([44, PBn], DT, tag="gate0", name="g_i2")
        nc.scalar.activation(g_i2[:], ps_i2[0:44, :PBn], AF.Sigmoid,
                             bias=bias_sb[0:44, 15:16])
        ps_o2 = pool_ps.tile([128, PB_MAX], mybir.dt.float32, tag="ps", name="ps_o2")
        mms(ps_o2, 1, a, b, 0, None)
        g_o2 = gate_pool.tile([44, PBn], DT, tag="gate1", name="g_o2")
        nc.scalar.activation(g_o2[:], ps_o2[0:44, :PBn], AF.Sigmoid,
                             bias=bias_sb[64:108, 15:16])
        ps_fl = pool_ps.tile([128, PB_MAX], mybir.dt.float32, tag="ps", name="ps_fl2")
        mms(ps_fl, 3, a, b, 0, None)
        g_fl2 = gate_pool.tile([44, PBn], DT, tag="gate3", name="g_fl2")
        nc.scalar.activation(g_fl2[:], ps_fl[0:44, :PBn], AF.Sigmoid,
                             bias=bias_sb[0:44, 16:17])
        ps_fr = pool_ps.tile([128, PB_MAX], mybir.dt.float32, tag="ps", name="ps_fr2")
        mms(ps_fr, 4, a, b, 0, None)
        g_fr2 = gate_pool.tile([44, PBn], DT, tag="gate4", name="g_fr2")
        nc.scalar.activation(g_fr2[:], ps_fr[0:44, :PBn], AF.Sigmoid,
                             bias=bias_sb[64:108, 16:17])
        ps_u = pool_ps.tile([128, PB_MAX], mybir.dt.float32, tag="ps", name="ps_u2")
        mms(ps_u, 2, a, b, 0, None)
        g_u = gate_pool.tile([44, PBn], DT, tag="gate2", name="g_u2")
        nc.scalar.activation(g_u[:], ps_u[:44, :PBn], AF.Tanh,
                             bias=bias_sb[:44, 8:9])
        gates[0][2] = g_i2[:]
        gates[1][2] = g_o2[:]
        gates[2][2] = g_u[:]
        gates[3][2] = g_fl2[:]
        gates[4][2] = g_fr2[:]
        return gates

    def gemm_gates_fused(rhs_k, PBn, pool2, pool1, gpool, n_free=512):
        """Phase-B variant: the rhs chunk-2 carries a ones-row at partition 44
        so the GEMM adds the bias; ACTs merge same-function gate pairs.
        j0/j1: psum pair (i|o) [128, 2*PBn], pair (fL|fR), u [128, PBn].
        j2: the 44-wide chunks as col-tiled pairs (i2|o2), (fL2|fR2), u2."""
        gates = [[None] * NCH for _ in range(5)]

        def mm_into(ps, g, a, b, off, base=0, tile_pos=None):
            cw = b - a
            for n0 in range(0, PBn, n_free):
                n1 = min(n0 + n_free, PBn)
                for kc in range(5):
                    nc.tensor.matmul(
                        ps[base:base + cw, off + n0: off + n1],
                        wrec_sb[kc][:, 300 * g + a: 300 * g + b],
                        rhs_k[kc][:, n0:n1],
                        start=(kc == 0), stop=(kc == 4),
                        tile_position=tile_pos)

        for j, (a, b) in enumerate(FCH[:2]):
            cw = b - a
            ps_io = pool2.tile([128, 2 * PB_MAX], mybir.dt.float32, tag="ps2",
                               name="ps_io")
            mm_into(ps_io, 0, a, b, 0)
            mm_into(ps_io, 1, a, b, PBn)
            g_io = gpool.tile([cw, 2 * PBn], DT, tag="gate_io", name=f"g_io_{j}")
            nc.scalar.activation(g_io[:], ps_io[:cw, :2 * PBn], AF.Sigmoid)

            ps_f = pool2.tile([128, 2 * PB_MAX], mybir.dt.float32, tag="ps2",
                              name="ps_f")
            mm_into(ps_f, 3, a, b, 0)
            mm_into(ps_f, 4, a, b, PBn)
            g_f = gpool.tile([cw, 2 * PBn], DT, tag="gate_f", name=f"g_f_{j}")
            nc.scalar.activation(g_f[:], ps_f[:cw, :2 * PBn], AF.Sigmoid)

            ps_u = pool1.tile([128, PB_MAX], mybir.dt.float32, tag="ps1",
                              name="ps_u")
            mm_into(ps_u, 2, a, b, 0)
            g_u = gpool.tile([cw, PBn], DT, tag="gate_u", name=f"g_u_{j}")
            nc.scalar.activation(g_u[:], ps_u[:cw, :PBn], AF.Tanh)

            gates[0][j] = g_io[:, :PBn]
            gates[1][j] = g_io[:, PBn:]
            gates[2][j] = g_u[:]
            gates[3][j] = g_f[:, :PBn]
            gates[4][j] = g_f[:, PBn:]

        # j2: col-tiled pairs in one 2-bank psum tile (io2 | f2), u2 in ps1
        a, b = FCH[2]
        ps2 = pool2.tile([128, 2 * PB_MAX], mybir.dt.float32, tag="ps2",
                         name="ps_j2")
        mm_into(ps2, 0, a, b, 0, 0)
        mm_into(ps2, 3, a, b, PBn, 0)
        g_a = gpool.tile([44, 2 * PBn], DT, tag="gate_io", name="g_j2a")
        nc.scalar.activation(g_a[:], ps2[0:44, :2 * PBn], AF.Sigmoid)
        ps2b = pool2.tile([128, 2 * PB_MAX], mybir.dt.float32, tag="ps2",
                          name="ps_j2b")
        mm_into(ps2b, 1, a, b, 0, 0)
        mm_into(ps2b, 4, a, b, PBn, 0)
        g_b = gpool.tile([44, 2 * PBn], DT, tag="gate_f", name="g_j2b")
        nc.scalar.activation(g_b[:], ps2b[0:44, :2 * PBn], AF.Sigmoid)
        ps_u = pool1.tile([128, PB_MAX], mybir.dt.float32, tag="ps1", name="ps_u2")
        mm_into(ps_u, 2, a, b, 0)
        g_u2 = gpool.tile([44, PBn], DT, tag="gate_u", name="g_u2")
        nc.scalar.activation(g_u2[:], ps_u[:44, :PBn], AF.Tanh)
        gates[0][2] = g_a[:, :PBn]
        gates[1][2] = g_b[:, :PBn]
        gates[2][2] = g_u2[:]
        gates[3][2] = g_a[:, PBn:]
        gates[4][2] = g_b[:, PBn:]
        return gates

    def leaf_gemm_gates(x_k, PBn, psL_pool, n_free=512):
        """Leaf i,o,u gates from x chunks (3 K-chunks); j2 runs (i2|o2) as a
        col-tiled concurrent pair."""
        gates = [[None] * NCH for _ in range(3)]

        def mms(ps, g, a, b, base, tile_pos):
            cw = b - a
            for n0 in range(0, PBn, n_free):
                n1 = min(n0 + n_free, PBn)
                for kc in range(NCH):
                    nc.tensor.matmul(
                        ps[base:base + cw, n0:n1],
                        wleaf_sb[kc][:, 300 * g + a: 300 * g + b],
                        x_k[kc][:, n0:n1],
                        start=(kc == 0), stop=(kc == NCH - 1),
                        tile_position=tile_pos)

        for j, (a, b) in enumerate(FCH[:2]):
            for g in range(3):
                cw = b - a
                ps = psL_pool.tile([128, LB], mybir.dt.float32, tag="psL", name="psL")
                mms(ps, g, a, b, 0, None)
                gt = gate_pool.tile([cw, PBn], DT, tag=f"lgate{g}", name=f"lgate{g}_{j}")
                m = g * 3 + j
                nc.scalar.activation(gt[:], ps[:cw, :PBn], GATE_FUNC[g],
                                     bias=bias_sb[:cw, m:m + 1])
                gates[g][j] = gt
        a, b = FCH[2]
        ps_i2 = psL_pool.tile([128, LB], mybir.dt.float32, tag="psL", name="psL_i2")
        mms(ps_i2, 0, a, b, 0, None)
        lg_i2 = gate_pool.tile([44, PBn], DT, tag="lgate0", name="lg_i2")
        nc.scalar.activation(lg_i2[:], ps_i2[0:44, :PBn], AF.Sigmoid,
                             bias=bias_sb[0:44, 15:16])
        ps_o2 = psL_pool.tile([128, LB], mybir.dt.float32, tag="psL", name="psL_o2")
        mms(ps_o2, 1, a, b, 0, None)
        lg_o2 = gate_pool.tile([44, PBn], DT, tag="lgate1", name="lg_o2")
        nc.scalar.activation(lg_o2[:], ps_o2[0:44, :PBn], AF.Sigmoid,
                             bias=bias_sb[64:108, 15:16])
        ps_u = psL_pool.tile([128, LB], mybir.dt.float32, tag="psL", name="psL_u2")
        mms(ps_u, 2, a, b, 0, None)
        g_u = gate_pool.tile([44, PBn], DT, tag="lgate2", name="lg_u2")
        nc.scalar.activation(g_u[:], ps_u[:44, :PBn], AF.Tanh,
                             bias=bias_sb[:44, 8:9])
        gates[0][2] = lg_i2[:]
        gates[1][2] = lg_o2[:]
        gates[2][2] = g_u[:]
        return gates

    def write_split(dst_even, dst_odd, in0, in1, op):
        """out = in0 <op> in1, writing even-node cols to dst_even and odd-node
        cols to dst_odd (both dense [cw, PBn//2] APs). in0/in1 are block-dense
        [cw, PBn] APs."""
        cw, PBn = in0.shape[0], in0.shape[1]
        half = PBn // 2
        e0, o0 = _even_odd(in0, half)
        e1, o1 = _even_odd(in1, half)
        de = dst_even.rearrange("p (m b) -> p m b", b=B_LOC)
        do = dst_odd.rearrange("p (m b) -> p m b", b=B_LOC)
        nc.vector.tensor_tensor(de, e0, e1, op)
        nc.vector.tensor_tensor(do, o0, o1, op)

    MUL = mybir.AluOpType.mult
    ADD = mybir.AluOpType.add

    def l1_ap(tensors, rows_a, rows_b, gcol, width):
        t = gcol // 1024
        lc = gcol % 1024
        return tensors[t][rows_a:rows_b, lc:lc + width]

    def recur_elementwise(gates, CL, CR, PBn, c_dst, h_dst, h2_odd_dma,
                          split=True):
        """Elementwise part for one block of an internal level.

        gates[g][j]: [cw, PBn] block-dense.  CL[j]/CR[j]: dense [cw, PBn] child-C
        APs (left/right).  c_dst/h_dst: per-j (even_ap, odd_ap) pairs, each
        [cw, PBn//2] dense in this level's storage order.
        h2_odd_dma: None, or (tile_ap_dst) for the chunk2 odd half that must be
        DMA'd to partitions 64:108 of a packed tile.
        split=False (root level): c_dst/h_dst are (full_ap, None); no
        deinterleave is applied."""
        for j in range(NCH):
            cw = FCH[j][1] - FCH[j][0]
            t1 = tmp_pool.tile([cw, PBn], DT, tag="t1", name=f"t1_{j}")
            t2 = tmp_pool.tile([cw, PBn], DT, tag="t2", name=f"t2_{j}")
            nc.vector.tensor_tensor(t1[:], gates[3][j][:], CL[j], MUL)
            nc.vector.tensor_tensor(t2[:], gates[4][j][:], CR[j], MUL)
            fc = tmp_pool.tile([cw, PBn], DT, tag="fc", name=f"fc_{j}")
            nc.vector.tensor_tensor(fc[:], t1[:], t2[:], ADD)
            iu = tmp_pool.tile([cw, PBn], DT, tag="iu", name=f"iu_{j}")
            nc.vector.tensor_tensor(iu[:], gates[0][j][:], gates[2][j][:], MUL)
            ce, co, cfull = c_dst[j]
            he, ho = h_dst[j]
            th = tmp_pool.tile([cw, PBn], DT, tag="th", name=f"th_{j}")
            if not split:
                nc.vector.tensor_tensor(ce, iu[:], fc[:], ADD)
                nc.scalar.activation(th[:], cfull, AF.Tanh)
                nc.vector.tensor_tensor(he, gates[1][j][:], th[:], MUL)
                continue
            # c (split write into storage order)
            write_split(ce, co, iu[:], fc[:], ADD)
            # tanh(c) over the split-ordered pair (halves of one contiguous tile)
            half = PBn // 2
            nc.scalar.activation(th[:], cfull, AF.Tanh)
            # h = o * tanh(c): o is block-dense; th halves are storage-ordered
            e_o, o_o = _even_odd(gates[1][j][:], half)
            nc.vector.tensor_tensor(
                he.rearrange("p (m b) -> p m b", b=B_LOC), e_o,
                th[:, :half].rearrange("p (m b) -> p m b", b=B_LOC), MUL)
            if j == 2 and h2_odd_dma is not None:
                nc.vector.tensor_tensor(
                    h2_odd_dma.rearrange("p (m b) -> p m b", b=B_LOC), o_o,
                    th[:, half:].rearrange("p (m b) -> p m b", b=B_LOC), MUL)
            else:
                nc.vector.tensor_tensor(
                    ho.rearrange("p (m b) -> p m b", b=B_LOC), o_o,
                    th[:, half:].rearrange("p (m b) -> p m b", b=B_LOC), MUL)

    # ================================================================ phase A
    # leaves + level-1, fused per leaf block of LB columns
    n_lblk = XCOLS // LB                       # 8 blocks
    ctx_a = ExitStack()
    xpool = ctx_a.enter_context(tc.tile_pool(name="xpool", bufs=2))
    gate_pool = ctx_a.enter_context(tc.tile_pool(name="gatesA", bufs=2))
    leaf_pool = ctx_a.enter_context(tc.tile_pool(name="leafp", bufs=1))
    psL_pool = ctx_a.enter_context(tc.tile_pool(name="psL", bufs=2, space="PSUM"))
    psA_pool = ctx_a.enter_context(tc.tile_pool(name="psA", bufs=4, space="PSUM"))
    for blk in range(n_lblk):
        c0 = blk * LB
        # --- stage x (3 chunks; chunk2 padded to 128 with zeros)
        x_k = []
        for j, (a, b) in enumerate(FCH):
            cw = b - a
            t = xpool.tile([128, LB], DTR, tag=f"x{j}")
            if cw < 128:
                nc.sync.dma_start(t[cw:, :], zeros_d[46: 46 + 128 - cw, :LB])
            nc.sync.dma_start(t[:cw], xt_d[a:b, c0:c0 + LB])
            x_k.append(t)
        # --- leaf gates
        lg = leaf_gemm_gates(x_k, LB, psL_pool)
        # --- leaf elementwise -> leaf H/C (deinterleaved, block-local)
        half = LB // 2
        lh = [leaf_pool.tile([128, LB], DTR, tag="lh0", name="lh0"),
              leaf_pool.tile([128, LB], DTR, tag="lh1", name="lh1"),
              leaf_pool.tile([128, half], DTR, tag="lh2p", name="lh2p")]  # packed chunk2
        lc = [leaf_pool.tile([128, LB], DT, tag="lc0", name="lc0"),
              leaf_pool.tile([128, LB], DT, tag="lc1", name="lc1"),
              leaf_pool.tile([44, LB], DT, tag="lc2", name="lc2")]
        nc.sync.dma_start(lh[2][44:64, :], zeros_d[46:66, :half])
        nc.sync.dma_start(lh[2][108:128, :], zeros_d[46:66, :half])
        for j in range(NCH):
            cw = FCH[j][1] - FCH[j][0]
            # c = i * u, split write
            write_split(lc[j][:cw, :half], lc[j][:cw, half:],
                        lg[0][j][:], lg[2][j][:], MUL)
            th = tmp_pool.tile([cw, LB], DT, tag="lth", name=f"lth_{j}")
            nc.scalar.activation(th[:], lc[j][:cw, :], AF.Tanh)
            e_o, o_o = _even_odd(lg[1][j][:], half)
            if j == 2:
                nc.vector.tensor_tensor(
                    lh[2][:cw, :].rearrange("p (m b) -> p m b", b=B_LOC), e_o,
                    th[:, :half].rearrange("p (m b) -> p m b", b=B_LOC), MUL)
                nc.vector.tensor_tensor(
                    lh[2][64:64 + cw, :].rearrange("p (m b) -> p m b", b=B_LOC), o_o,
                    th[:, half:].rearrange("p (m b) -> p m b", b=B_LOC), MUL)
            else:
                nc.vector.tensor_tensor(
                    lh[j][:cw, :half].rearrange("p (m b) -> p m b", b=B_LOC), e_o,
                    th[:, :half].rearrange("p (m b) -> p m b", b=B_LOC), MUL)
                nc.vector.tensor_tensor(
                    lh[j][:cw, half:].rearrange("p (m b) -> p m b", b=B_LOC), o_o,
                    th[:, half:].rearrange("p (m b) -> p m b", b=B_LOC), MUL)

        # --- level-1 for this block: PBn = LB//2 parent cols
        PBn = half                             # 512
        rhs_k = [lh[0][:, :PBn], lh[1][:, :PBn], lh[2][:, :PBn],
                 lh[0][:, PBn:PBn * 2], lh[1][:, PBn:PBn * 2]]
        gates = gemm_gates(rhs_k, PBn, pool_ps=psA_pool)
        CLs = [lc[0][:128, :PBn], lc[1][:128, :PBn], lc[2][:44, :PBn]]
        CRs = [lc[0][:128, PBn:], lc[1][:128, PBn:], lc[2][:44, PBn:]]
        # destination: DRAM level 1, parent cols [blk*PBn, (blk+1)*PBn)
        p0 = blk * PBn
        oc = [out_pool.tile([FCH[j][1] - FCH[j][0], PBn], DT, tag=f"oc{j}", name=f"oc{j}")
              for j in range(NCH)]
        oh = [out_pool.tile([FCH[j][1] - FCH[j][0], PBn], DTR, tag=f"oh{j}", name=f"oh{j}")
              for j in range(NCH)]
        qh = PBn // 2
        c_dst = [(oc[j][:, :qh], oc[j][:, qh:], oc[j][:, :]) for j in range(NCH)]
        h_dst = [(oh[j][:, :qh], oh[j][:, qh:]) for j in range(NCH)]
        recur_elementwise(gates, CLs, CRs, PBn, c_dst, h_dst, None)
        R1 = R_LVL[0]
        for j, (a, b) in enumerate(FCH):
            nc.sync.dma_start(l1_ap(Cd[1], a, b, p0 // 2, qh), oc[j][:, :qh])
            nc.sync.dma_start(l1_ap(Cd[1], a, b, R1 // 2 + p0 // 2, qh),
                              oc[j][:, qh:])
            nc.sync.dma_start(l1_ap(Hd[1], a, b, p0 // 2, qh), oh[j][:, :qh])
            nc.sync.dma_start(l1_ap(Hd[1], a, b, R1 // 2 + p0 // 2, qh),
                              oh[j][:, qh:])

    ctx_a.close()

    # ================================================================ phase B
    # levels 2..9
    ctx_b = ExitStack()
    stage_pool = ctx_b.enter_context(tc.tile_pool(name="stage", bufs=2))
    psB2_pool = ctx_b.enter_context(tc.tile_pool(name="psB2", bufs=3, space="PSUM"))
    gateB_pool = ctx_b.enter_context(tc.tile_pool(name="gatesB", bufs=3))
    psB1_pool = ctx_b.enter_context(tc.tile_pool(name="psB1", bufs=2, space="PSUM"))
    for lvl in range(2, 10):
        R = R_LVL[lvl - 1]          # this level's column count
        Rp = R_LVL[lvl - 2]         # previous level's column count
        PBn = min(PB_MAX, R)
        prev_dram = (lvl - 1) <= N_DRAM_LVLS
        this_dram = lvl <= N_DRAM_LVLS
        for blk in range(R // PBn):
            p0 = blk * PBn
            # ---- children staging / APs
            if prev_dram:
                if lvl - 1 == 1:
                    def h_src(ra, rb, gc, w):
                        return l1_ap(Hd[1], ra, rb, gc, w)
                    def c_src(ra, rb, gc, w):
                        return l1_ap(Cd[1], ra, rb, gc, w)
                else:
                    def h_src(ra, rb, gc, w):
                        return Hd[lvl - 1][ra:rb, gc:gc + w]
                    def c_src(ra, rb, gc, w):
                        return Cd[lvl - 1][ra:rb, gc:gc + w]
                s0 = stage_pool.tile([128, 2 * PBn], DTR, tag="s0", name="s0")
                s1 = stage_pool.tile([128, 2 * PBn], DTR, tag="s1", name="s1")
                s2 = stage_pool.tile([128, PBn], DTR, tag="s2p", name="s2p")
                nc.sync.dma_start(s2[44:64, :], zeros_d[44:64, :PBn])
                nc.sync.dma_start(s2[108:128, :], zeros_d[46:66, :PBn])
                nc.sync.dma_start(s0[:, :PBn], h_src(0, 128, p0, PBn))
                nc.sync.dma_start(s0[:, PBn:], h_src(0, 128, Rp // 2 + p0, PBn))
                nc.sync.dma_start(s1[:, :PBn], h_src(128, 256, p0, PBn))
                nc.sync.dma_start(s1[:, PBn:], h_src(128, 256, Rp // 2 + p0, PBn))
                nc.sync.dma_start(s2[0:44, :], h_src(256, 300, p0, PBn))
                nc.sync.dma_start(s2[64:108, :], h_src(256, 300, Rp // 2 + p0, PBn))
                sc = []
                for j, (a, b) in enumerate(FCH):
                    cw = b - a
                    t = stage_pool.tile([cw, 2 * PBn], DT, tag=f"sc{j}", name=f"sc{j}")
                    nc.sync.dma_start(t[:, :PBn], c_src(a, b, p0, PBn))
                    nc.sync.dma_start(t[:, PBn:], c_src(a, b, Rp // 2 + p0, PBn))
                    sc.append(t)
                rhs_k = [s0[:, :PBn], s1[:, :PBn], s2[:, :],
                         s0[:, PBn:], s1[:, PBn:]]
                CLs = [sc[j][:, :PBn] for j in range(NCH)]
                CRs = [sc[j][:, PBn:] for j in range(NCH)]
            else:
                h0, h1, h2 = Hsb[lvl - 1]
                cc0, cc1, cc2 = Csb[lvl - 1]
                hw = Rp // 2     # == PBn for single-block levels
                rhs_k = [h0[:, :hw], h1[:, :hw], h2[:, :],
                         h0[:, hw:], h1[:, hw:]]
                CLs = [cc0[:128, :hw], cc1[:128, :hw], cc2[:44, :hw]]
                CRs = [cc0[:128, hw:], cc1[:128, hw:], cc2[:44, hw:]]

            gates = gemm_gates_fused(rhs_k, PBn, psB2_pool, psB1_pool, gateB_pool)

            # ---- destinations
            qh = PBn // 2
            if lvl == 9:
                # root: single node -> no deinterleave; write dense output tiles
                oc = [out_pool.tile([FCH[j][1] - FCH[j][0], PBn], DT, tag=f"oc{j}", name=f"oc{j}")
                      for j in range(NCH)]
                oh = [out_pool.tile([FCH[j][1] - FCH[j][0], PBn], DT, tag=f"ohr{j}", name=f"ohr{j}")
                      for j in range(NCH)]
                c_dst = [(oc[j][:, :PBn], None, oc[j][:, :PBn]) for j in range(NCH)]
                h_dst = [(oh[j][:, :PBn], None) for j in range(NCH)]
                recur_elementwise(gates, CLs, CRs, PBn, c_dst, h_dst, None,
                                  split=False)
                for j, (a, b) in enumerate(FCH):
                    nc.sync.dma_start(out_d[0, a:b, :], oc[j][:])
                    nc.sync.dma_start(out_d[1, a:b, :], oh[j][:])
            elif this_dram:
                oc = [out_pool.tile([FCH[j][1] - FCH[j][0], PBn], DT, tag=f"oc{j}", name=f"oc{j}")
                      for j in range(NCH)]
                oh = [out_pool.tile([FCH[j][1] - FCH[j][0], PBn], DTR, tag=f"oh{j}", name=f"oh{j}")
                      for j in range(NCH)]
                c_dst = [(oc[j][:, :qh], oc[j][:, qh:], oc[j][:, :]) for j in range(NCH)]
                h_dst = [(oh[j][:, :qh], oh[j][:, qh:]) for j in range(NCH)]
                recur_elementwise(gates, CLs, CRs, PBn, c_dst, h_dst, None)
                for j, (a, b) in enumerate(FCH):
                    nc.sync.dma_start(Cd[lvl][a:b, p0 // 2: p0 // 2 + qh], oc[j][:, :qh])
                    nc.sync.dma_start(Cd[lvl][a:b, R // 2 + p0 // 2: R // 2 + p0 // 2 + qh],
                                      oc[j][:, qh:])
                    nc.sync.dma_start(Hd[lvl][a:b, p0 // 2: p0 // 2 + qh], oh[j][:, :qh])
                    nc.sync.dma_start(Hd[lvl][a:b, R // 2 + p0 // 2: R // 2 + p0 // 2 + qh],
                                      oh[j][:, qh:])
            else:
                # SBUF-resident destination (single block: p0 == 0)
                h0, h1, h2 = Hsb[lvl]
                cc0, cc1, cc2 = Csb[lvl]
                hh = R // 2
                c_dst = [(cc0[:128, :hh], cc0[:128, hh:], cc0[:128, :]),
                         (cc1[:128, :hh], cc1[:128, hh:], cc1[:128, :]),
                         (cc2[:44, :hh], cc2[:44, hh:], cc2[:44, :])]
                h_dst = [(h0[:128, :hh], h0[:128, hh:]),
                         (h1[:128, :hh], h1[:128, hh:]),
                         (h2[0:44, :], None)]   # odd half goes via DMA
                recur_elementwise(gates, CLs, CRs, PBn, c_dst, h_dst,
                                  h2[64:108, :])
    ctx_b.close()


# ---------------------------------------------------------------- runner
_CACHE = {}


def _get_program():
    if "nc" not in _CACHE:
        _CACHE["nc"] = build_program()
    return _CACHE["nc"]


def kernel(inputs, Wfioux, b_fioux, Wiouh, Wfh, left_idx, right_idx, leaf_mask,
           _trace=False, _trace_dir=None):
    inputs = np.asarray(inputs, np.float32)
    assert _check_topology(left_idx, right_idx, leaf_mask), \
        "tree topology does not match the expected complete binary tree"

    wleaf, wrec, biasT = _pack_weights(
        np.asarray(Wfioux, np.float32), np.asarray(b_fioux, np.float32),
        np.asarray(Wiouh, np.float32), np.asarray(Wfh, np.float32))

    in_maps = []
    for core in range(N_CORES):
        x = inputs[core * B_LOC:(core + 1) * B_LOC, :N_LEAVES, :]
        xt = np.ascontiguousarray(x.transpose(2, 1, 0).reshape(MEM, XCOLS))
        in_maps.append({"xt": xt, "wleaf": wleaf, "wrec": wrec, "biasT": biasT,
                        "consts": _consts()})

    nc = _get_program()
    res = run_bass_kernel_spmd(nc, in_maps, list(range(N_CORES)),
                               trace=_trace, tmpdir=_trace_dir)

    root_c = np.zeros((B, MEM), np.float32)
    root_h = np.zeros((B, MEM), np.float32)
    for core in range(N_CORES):
        out = res.results[core]["out"]          # [2, 300, 16]
        root_c[core * B_LOC:(core + 1) * B_LOC] = out[0].T
        root_h[core * B_LOC:(core + 1) * B_LOC] = out[1].T
    _CACHE["last_results"] = res
    return root_c, root_h



# revision 9
# speedup vs baseline: 1.2206x; 1.2206x over previous
"""Trainium2 Bass kernel for a batch-of-trees BinaryTreeLSTM (fp16 rewrite).

Contract: kernel(**inputs) takes the FULL inputs (B=128 trees, 1023-node
complete binary tree, dim 300) and returns the FULL output (root_c, root_h),
each [128, 300] float32.

Strategy
--------
- Data-parallel over trees: 16 trees per NeuronCore x 8 cores, no collectives.
- fp16 everywhere: GEMM operands (weights, x, h), gates, c/h states.  fp32
  PSUM accumulate + fp32 ACT internals keep the root error ~1e-3 (emulated),
  well under the 2e-2 gate.  fp16 runs 1 cycle/row on the PE at ANY free size
  (fp32r pays 4x below 256), halves LDWEIGHTS, DMA and SBUF vs fp32r.
- M-repacked gate units: the matmul cost model is (#units x #K-chunks) x N,
  independent of unit row-width, so the 1500 recurrent gate rows (i,o,u,fL,fR
  x 300) are packed into 12 units of <=128 rows (vs 15 naive) and the 900
  leaf gate rows into 8 units (vs 9).  Tail rows of several gates share units.
- Bias enters via a ones-row at partition 44 of the packed K-chunk 2 (the
  chunk that carries child-h features 256:300 of left/right at partitions
  0:44 / 64:108), so ACT applies pure sigmoid/tanh and pairs of units merge
  into single wide ACT instructions.
- All state is SBUF-resident (fp16 makes it fit); no DRAM round-trips for
  levels 1..3 anymore.  States stored deinterleaved ([even nodes | odd]) per
  feature chunk: h01/c01 [128, 2R] (chunks 0,1), h2p/c2p [128, R/2] with the
  44-row chunk-2 packed at partitions 0:44 (even) / 64:108 (odd).
- Phase A software-pipelines leaf blocks against the previous block's level-1
  GEMM so the PE never waits on the leaf elementwise chain.
"""

import os
import sys

for _p in ("/opt/trn_rl_repo",):
    if os.path.isdir(_p) and _p not in sys.path:
        sys.path.insert(0, _p)

import numpy as np
from contextlib import ExitStack

import concourse.bass as bass
import concourse.tile as tile
from concourse import bacc, mybir
from concourse.bass_utils import run_bass_kernel_spmd

# ---------------------------------------------------------------- constants
N_CORES = 8
B = 128
B_LOC = B // N_CORES          # 16 trees per core
N_LEAVES = 512
MEM = 300
XCOLS = N_LEAVES * B_LOC      # 8192 leaf columns per core
LB = 1024                     # leaf-block columns (64 leaves)
NF = 512                      # max moving free dim
R_LVL = {l: XCOLS >> l for l in range(1, 10)}   # level l column count

F16 = mybir.dt.float16
F32 = mybir.dt.float32
AF = mybir.ActivationFunctionType
SIG = AF.Sigmoid
TANH = AF.Tanh
MUL = mybir.AluOpType.mult
ADD = mybir.AluOpType.add

# Leaf M-units (8 units over Wiou cols [i 0:300 | o 300:600 | u 600:900]):
# unit -> list of (dst_row0, dst_row1, src_col0)
# (SBUF compute APs must start at partition 0 or 64, so tail gates sit at
# those offsets; rows 44:64 / 108:128 of tail units carry zero weights.)
LEAF_SLOTS = {
    0: [(0, 128, 0)], 1: [(0, 128, 128)],
    2: [(0, 128, 300)], 3: [(0, 128, 428)],
    4: [(0, 128, 600)], 5: [(0, 128, 728)],
    6: [(0, 44, 256), (64, 108, 556)],   # i2 @0 | o2 @64
    7: [(0, 44, 856)],                   # u2 @0
}
# Recurrent M-units (13 units over Wcat cols
# [i 0:300 | o 300:600 | u 600:900 | fL 900:1200 | fR 1200:1500]):
REC_SLOTS = {
    0: [(0, 128, 0)], 1: [(0, 128, 128)],
    2: [(0, 128, 300)], 3: [(0, 128, 428)],
    4: [(0, 128, 900)], 5: [(0, 128, 1028)],     # fL
    6: [(0, 128, 1200)], 7: [(0, 128, 1328)],    # fR
    8: [(0, 128, 600)], 9: [(0, 128, 728)],      # u
    10: [(0, 44, 256), (64, 108, 556)],          # i2 @0 | o2 @64
    11: [(0, 44, 856), (64, 108, 1156)],         # u2 @0 | fL2 @64
    12: [(0, 44, 1456)],                         # fR2 @0
}


# ---------------------------------------------------------------- host packing
def _pack_weights(Wfioux, b_fioux, Wiouh, Wfh):
    f4 = np.float32
    Wiou = np.asarray(Wfioux[:, 300:1200], f4)            # [300, 900]
    bleaf = np.asarray(b_fioux[300:1200], f4)             # [900]
    wleaf = np.zeros((3, 128, 8 * 128), f4)
    kch_l = [(0, 128), (128, 256), (256, 300)]
    for kc, (ra, rb) in enumerate(kch_l):
        for m, slots in LEAF_SLOTS.items():
            for (r0, r1, c0) in slots:
                wleaf[kc, 0: rb - ra, 128 * m + r0: 128 * m + r1] = \
                    Wiou[ra:rb, c0: c0 + (r1 - r0)]
    # bias via ones-row at partition 44 of K-chunk 2
    for m, slots in LEAF_SLOTS.items():
        for (r0, r1, c0) in slots:
            wleaf[2, 44, 128 * m + r0: 128 * m + r1] = bleaf[c0: c0 + (r1 - r0)]

    Wcat = np.concatenate([Wiouh, Wfh], axis=1).astype(f4)  # [600, 1500]
    bf = np.asarray(b_fioux[0:300], f4)
    bias_cat = np.concatenate(
        [b_fioux[300:600], b_fioux[600:900], b_fioux[900:1200], bf, bf]
    ).astype(f4)
    wrec = np.zeros((5, 128, 13 * 128), f4)
    # K-chunks: 0: hL[0:128], 1: hL[128:256], 2: packed hL[256:300]@0:44 +
    # ones@44 + hR[256:300]@64:108, 3: hR[0:128], 4: hR[128:256]
    kch_r = [(0, 128, 0), (128, 256, 0), None, (300, 428, 0), (428, 556, 0)]
    for kc, span in enumerate(kch_r):
        if span is None:
            continue
        ra, rb, _ = span
        for m, slots in REC_SLOTS.items():
            for (r0, r1, c0) in slots:
                wrec[kc, 0: rb - ra, 128 * m + r0: 128 * m + r1] = \
                    Wcat[ra:rb, c0: c0 + (r1 - r0)]
    for m, slots in REC_SLOTS.items():
        for (r0, r1, c0) in slots:
            wrec[2, 0:44, 128 * m + r0: 128 * m + r1] = \
                Wcat[256:300, c0: c0 + (r1 - r0)]
            wrec[2, 44, 128 * m + r0: 128 * m + r1] = bias_cat[c0: c0 + (r1 - r0)]
            wrec[2, 64:108, 128 * m + r0: 128 * m + r1] = \
                Wcat[556:600, c0: c0 + (r1 - r0)]
    return wleaf.astype(np.float16), wrec.astype(np.float16)


def _check_topology(left_idx, right_idx, leaf_mask):
    li = np.asarray(left_idx); ri = np.asarray(right_idx)
    prev = np.arange(N_LEAVES); nid = N_LEAVES
    ok = bool((np.asarray(leaf_mask)[:N_LEAVES] == 1).all())
    ok &= bool((np.asarray(leaf_mask)[N_LEAVES:] == 0).all())
    while len(prev) > 1:
        cur = []
        for k in range(0, len(prev), 2):
            ok &= bool(li[nid] == prev[k]) and bool(ri[nid] == prev[k + 1])
            cur.append(nid); nid += 1
        prev = np.asarray(cur)
    return ok


# ---------------------------------------------------------------- bass program
def _ev_od(ap, b=B_LOC):
    """Block-dense [p, X] (node-major, X = m*2*b) -> (even, odd) [p, m, b]."""
    r = ap.rearrange("p (m two b) -> p m two b", two=2, b=b)
    return r[:, :, 0, :], r[:, :, 1, :]


def _mb(ap, b=B_LOC):
    return ap.rearrange("p (m b) -> p m b", b=b)


def build_program():
    nc = bacc.Bacc("TRN2", target_bir_lowering=False, debug=False)

    xt_d = nc.dram_tensor("xt", [MEM, XCOLS], F16, kind="ExternalInput").ap()
    wleaf_d = nc.dram_tensor("wleaf", [3, 128, 8 * 128], F16,
                             kind="ExternalInput").ap()
    wrec_d = nc.dram_tensor("wrec", [5, 128, 13 * 128], F16,
                            kind="ExternalInput").ap()
    cons_d = nc.dram_tensor("cons", [84, 2 * LB], F16, kind="ExternalInput").ap()
    out_d = nc.dram_tensor("out", [2, MEM, B_LOC], F16, kind="ExternalOutput").ap()

    with ExitStack() as ctx:
        tc = ctx.enter_context(tile.TileContext(nc))
        _build(ctx, tc, xt_d, wleaf_d, wrec_d, cons_d, out_d)

    nc.compile()
    return nc


def _build(ctx, tc, xt_d, wleaf_d, wrec_d, cons_d, out_d):
    nc = tc.nc

    wpool = ctx.enter_context(tc.tile_pool(name="wpool", bufs=1))
    state_pool = ctx.enter_context(tc.tile_pool(name="state", bufs=1))

    # ---- weights resident in SBUF (leaf weights first: needed immediately)
    wleaf_sb = []
    for k in range(3):
        t = wpool.tile([128, 8 * 128], F16, name=f"wleaf{k}")
        nc.sync.dma_start(t[:], wleaf_d[k])
        wleaf_sb.append(t)
    wrec_sb = [wpool.tile([128, 13 * 128], F16, name=f"wrec{k}") for k in range(5)]

    # ---- persistent SBUF state for levels 1..8
    ST = {}
    for lvl in range(1, 9):
        R = R_LVL[lvl]
        h01 = state_pool.tile([128, 2 * R], F16, name=f"h01_{lvl}")
        h2p = state_pool.tile([128, R // 2], F16, name=f"h2p_{lvl}")
        c01 = state_pool.tile([128, 2 * R], F16, name=f"c01_{lvl}")
        c2p = state_pool.tile([128, R // 2], F16, name=f"c2p_{lvl}")
        nc.sync.dma_start(h2p[44:64, :], cons_d[0:20, : R // 2])  # ones@44
        nc.sync.dma_start(h2p[108:128, :], cons_d[1:21, : R // 2])
        ST[lvl] = dict(h01=h01, h2p=h2p, c01=c01, c2p=c2p, R=R)

    # persistent double-buffered leaf tiles that carry constant rows
    x2_t = []
    lh2p_t = []
    for i in range(2):
        t = state_pool.tile([128, LB], F16, name=f"x2_{i}")
        nc.sync.dma_start(t[44:128, :], cons_d[0:84, :LB])   # ones@44, 0 below
        x2_t.append(t)
        t = state_pool.tile([128, LB // 2], F16, name=f"lh2p_{i}")
        nc.sync.dma_start(t[44:64, :], cons_d[0:20, : LB // 2])
        nc.sync.dma_start(t[108:128, :], cons_d[1:21, : LB // 2])
        lh2p_t.append(t)

    # ---- pools
    xpool = ctx.enter_context(tc.tile_pool(name="xpool", bufs=2))
    glpool = ctx.enter_context(tc.tile_pool(name="gl", bufs=2))
    lpool = ctx.enter_context(tc.tile_pool(name="lpool", bufs=2))
    gpool = ctx.enter_context(tc.tile_pool(name="g", bufs=2))
    pspool = ctx.enter_context(tc.tile_pool(name="ps", bufs=4, space="PSUM"))
    tmp = ctx.enter_context(tc.tile_pool(name="tmp", bufs=2))
    opool = ctx.enter_context(tc.tile_pool(name="o", bufs=1))

    # ================================================================ helpers
    def leaf_gemm(xk, s, Gl):
        """Leaf gates for sub-chunk s (512 cols): 4 psum pairs, 6 ACTs."""
        n0 = s * NF
        for pi in range(4):
            ua, ub = 2 * pi, 2 * pi + 1
            ps = pspool.tile([128, 2 * NF], F32, tag="ps", name=f"psl{pi}")
            for j, u in enumerate((ua, ub)):
                rows = (128, 128, 128, 128, 128, 128, 108, 44)[u]
                off = j * NF
                for kc in range(3):
                    nc.tensor.matmul(
                        ps[0:rows, off: off + NF],
                        wleaf_sb[kc][:, 128 * u: 128 * u + rows],
                        xk[kc][:, n0: n0 + NF],
                        start=(kc == 0), stop=(kc == 2))
            if pi < 3:
                func = SIG if pi < 2 else TANH
                nc.scalar.activation(Gl[:, 2 * pi * NF: (2 * pi + 2) * NF],
                                     ps[:, :], func)
            else:
                nc.scalar.activation(Gl[0:108, 6 * NF: 7 * NF],
                                     ps[0:108, 0:NF], SIG)
                nc.scalar.activation(Gl[0:44, 7 * NF: 8 * NF],
                                     ps[0:44, NF: NF + NF], TANH)

    def rec_gemm(rhs_k, PB, G):
        """Recurrent gates for one block of PB cols: 6 psum pairs + 1 single."""
        UROWS = (128,) * 10 + (108, 108, 44)
        for pi in range(7):
            units = (2 * pi, 2 * pi + 1) if pi < 6 else (12,)
            ps = pspool.tile([128, 2 * NF], F32, tag="ps", name=f"psr{pi}")
            for j, u in enumerate(units):
                rows = UROWS[u]
                off = j * PB
                for kc in range(5):
                    nc.tensor.matmul(
                        ps[0:rows, off: off + PB],
                        wrec_sb[kc][:, 128 * u: 128 * u + rows],
                        rhs_k[kc],
                        start=(kc == 0), stop=(kc == 4))
            if pi < 5:
                func = SIG if pi < 4 else TANH
                nc.scalar.activation(G[:, 2 * pi * PB: (2 * pi + 2) * PB],
                                     ps[:, 0: 2 * PB], func)
            elif pi == 5:
                # T10 = [i2@0 | o2@64] all sigmoid; T11 = [u2@0 | fL2@64]
                nc.scalar.activation(G[0:108, 10 * PB: 11 * PB],
                                     ps[0:108, 0:PB], SIG)
                nc.scalar.activation(G[0:44, 11 * PB: 12 * PB],
                                     ps[0:44, PB: 2 * PB], TANH)
                nc.scalar.activation(G[64:108, 11 * PB: 12 * PB],
                                     ps[64:108, PB: 2 * PB], SIG)
            else:
                nc.scalar.activation(G[0:44, 12 * PB: 13 * PB],
                                     ps[0:44, 0:PB], SIG)

    def st_sl(t, R, ch, eo, q0, w):
        off = ch * R + eo * (R // 2) + q0
        return t[:, off: off + w]

    def rec_ew(G, PB, CL, CR, dst, p0):
        """Elementwise for a recurrent block. G gates [128, 12*PB].
        CL/CR: (c0, c1, c2) child-c dense APs [.,PB] (c2: 44 rows).
        dst: ST[lvl] dict, or ('root', oc01, oc2, oh01, oh2) for level 9."""
        N = PB
        gi = [G[:, 0:N], G[:, N: 2 * N], G[0:44, 10 * N: 11 * N]]
        go = [G[:, 2 * N: 3 * N], G[:, 3 * N: 4 * N], G[64:108, 10 * N: 11 * N]]
        gfL = [G[:, 4 * N: 5 * N], G[:, 5 * N: 6 * N], G[64:108, 11 * N: 12 * N]]
        gfR = [G[:, 6 * N: 7 * N], G[:, 7 * N: 8 * N], G[0:44, 12 * N: 13 * N]]
        gu = [G[:, 8 * N: 9 * N], G[:, 9 * N: 10 * N], G[0:44, 11 * N: 12 * N]]

        t1 = tmp.tile([128, 2 * NF], F16, tag="t1", name="t1")
        t2 = tmp.tile([128, 2 * NF], F16, tag="t2", name="t2")
        fc = tmp.tile([128, 2 * NF], F16, tag="fc", name="fc")
        iu = tmp.tile([128, 2 * NF], F16, tag="iu", name="iu")
        t1_2 = tmp.tile([44, NF], F16, tag="t1_2", name="t1_2")
        t2_2 = tmp.tile([44, NF], F16, tag="t2_2", name="t2_2")
        fc2 = tmp.tile([44, NF], F16, tag="fc2", name="fc2")
        iu2 = tmp.tile([44, NF], F16, tag="iu2", name="iu2")

        for ch in range(2):
            nc.vector.tensor_tensor(t1[:, ch * N: (ch + 1) * N], gfL[ch],
                                    CL[ch], MUL)
            nc.vector.tensor_tensor(t2[:, ch * N: (ch + 1) * N], gfR[ch],
                                    CR[ch], MUL)
        nc.vector.tensor_tensor(t1_2[:, :N], gfL[2], CL[2], MUL)
        nc.vector.tensor_tensor(t2_2[:, :N], gfR[2], CR[2], MUL)
        nc.vector.tensor_tensor(fc[:, : 2 * N], t1[:, : 2 * N], t2[:, : 2 * N],
                                ADD)
        nc.vector.tensor_tensor(fc2[:, :N], t1_2[:, :N], t2_2[:, :N], ADD)
        nc.vector.tensor_tensor(iu[:, :N], gi[0], gu[0], MUL)
        nc.vector.tensor_tensor(iu[:, N: 2 * N], gi[1], gu[1], MUL)
        nc.vector.tensor_tensor(iu2[:, :N], gi[2], gu[2], MUL)

        if isinstance(dst, tuple) and dst[0] == "root":
            _, oc01, oc2, oh01, oh2 = dst
            nc.vector.tensor_tensor(oc01[:, : 2 * N], iu[:, : 2 * N],
                                    fc[:, : 2 * N], ADD)
            nc.vector.tensor_tensor(oc2[:, :N], iu2[:, :N], fc2[:, :N], ADD)
            th = tmp.tile([128, 2 * NF], F16, tag="th", name="th")
            th2 = tmp.tile([128, NF], F16, tag="th2", name="th2")
            nc.scalar.activation(th[:, : 2 * N], oc01[:, : 2 * N], TANH)
            nc.scalar.activation(th2[64:108, :N], oc2[:, :N], TANH)
            nc.vector.tensor_tensor(oh01[:, :N], go[0], th[:, :N], MUL)
            nc.vector.tensor_tensor(oh01[:, N: 2 * N], go[1], th[:, N: 2 * N],
                                    MUL)
            nc.vector.tensor_tensor(oh2[:, :N], go[2], th2[64:108, :N], MUL)
            return

        st = dst
        R = st["R"]
        q0, hw = p0 // 2, PB // 2
        # c writes (deinterleave into state), then tanh, then h writes
        for ch in range(2):
            iue, iuo = _ev_od(iu[:, ch * N: (ch + 1) * N])
            fce, fco = _ev_od(fc[:, ch * N: (ch + 1) * N])
            nc.vector.tensor_tensor(_mb(st_sl(st["c01"], R, ch, 0, q0, hw)),
                                    iue, fce, ADD)
            nc.vector.tensor_tensor(_mb(st_sl(st["c01"], R, ch, 1, q0, hw)),
                                    iuo, fco, ADD)
        iue, iuo = _ev_od(iu2[:, :N])
        fce, fco = _ev_od(fc2[:, :N])
        nc.vector.tensor_tensor(_mb(st["c2p"][64:108, q0: q0 + hw]), iue, fce,
                                ADD)
        nc.vector.tensor_tensor(_mb(st["c2p"][0:44, q0: q0 + hw]), iuo, fco,
                                ADD)

        # th layout: [ch0E | ch1E | ch0O | ch1O], each hw wide
        th = tmp.tile([128, 2 * NF], F16, tag="th", name="th")
        th2 = tmp.tile([128, NF], F16, tag="th2", name="th2")
        c4 = st["c01"].rearrange("p (ch eo q) -> p ch eo q", ch=2, eo=2)
        tho = th[:, : 2 * N].rearrange("p (eo ch q) -> p eo ch q", eo=2, ch=2)
        nc.scalar.activation(tho[:, 0], c4[:, :, 0, q0: q0 + hw], TANH)
        nc.scalar.activation(tho[:, 1], c4[:, :, 1, q0: q0 + hw], TANH)
        nc.scalar.activation(th2[64:108, 0:hw], st["c2p"][64:108, q0: q0 + hw],
                             TANH)
        nc.scalar.activation(th2[64:108, hw:N], st["c2p"][0:44, q0: q0 + hw],
                             TANH)

        for ch in range(2):
            oe, oo = _ev_od(go[ch])
            nc.vector.tensor_tensor(_mb(st_sl(st["h01"], R, ch, 0, q0, hw)),
                                    oe, _mb(th[:, ch * hw: (ch + 1) * hw]), MUL)
            nc.vector.tensor_tensor(
                _mb(st_sl(st["h01"], R, ch, 1, q0, hw)), oo,
                _mb(th[:, N + ch * hw: N + (ch + 1) * hw]), MUL)
        oe, oo = _ev_od(go[2])
        nc.vector.tensor_tensor(_mb(st["h2p"][0:44, q0: q0 + hw]), oe,
                                _mb(th2[64:108, 0:hw]), MUL)
        nc.vector.tensor_tensor(_mb(st["h2p"][64:108, q0: q0 + hw]), oo,
                                _mb(th2[64:108, hw:N]), MUL)

    def leaf_ew(Gl, s, lh01, lh2p, lc01, lc2p):
        """Leaf elementwise for sub-chunk s (512 cols): c = i*u, h = o*tanh(c).
        Writes deinterleaved into the LB-wide block-local leaf tiles."""
        N = NF
        q0, hw = s * (NF // 2), NF // 2
        gi = [Gl[:, 0:N], Gl[:, N: 2 * N], Gl[0:44, 6 * N: 7 * N]]
        go = [Gl[:, 2 * N: 3 * N], Gl[:, 3 * N: 4 * N], Gl[64:108, 6 * N: 7 * N]]
        gu = [Gl[:, 4 * N: 5 * N], Gl[:, 5 * N: 6 * N], Gl[0:44, 7 * N: 8 * N]]

        for ch in range(2):
            ie, io = _ev_od(gi[ch])
            ue, uo = _ev_od(gu[ch])
            nc.vector.tensor_tensor(_mb(st_sl(lc01, LB, ch, 0, q0, hw)), ie, ue,
                                    MUL)
            nc.vector.tensor_tensor(_mb(st_sl(lc01, LB, ch, 1, q0, hw)), io, uo,
                                    MUL)
        i2e, i2o = _ev_od(gi[2])
        u2e, u2o = _ev_od(gu[2])
        nc.vector.tensor_tensor(_mb(lc2p[64:108, q0: q0 + hw]), i2e, u2e, MUL)
        nc.vector.tensor_tensor(_mb(lc2p[0:44, q0: q0 + hw]), i2o, u2o, MUL)

        th = tmp.tile([128, 2 * NF], F16, tag="th", name="lth")
        th2 = tmp.tile([128, NF], F16, tag="th2", name="lth2")
        c4 = lc01.rearrange("p (ch eo q) -> p ch eo q", ch=2, eo=2)
        tho = th[:, : 2 * N].rearrange("p (eo ch q) -> p eo ch q", eo=2, ch=2)
        nc.scalar.activation(tho[:, 0], c4[:, :, 0, q0: q0 + hw], TANH)
        nc.scalar.activation(tho[:, 1], c4[:, :, 1, q0: q0 + hw], TANH)
        nc.scalar.activation(th2[64:108, 0:hw], lc2p[64:108, q0: q0 + hw],
                             TANH)
        nc.scalar.activation(th2[64:108, hw:N], lc2p[0:44, q0: q0 + hw], TANH)

        for ch in range(2):
            oe, oo = _ev_od(go[ch])
            nc.vector.tensor_tensor(_mb(st_sl(lh01, LB, ch, 0, q0, hw)), oe,
                                    _mb(th[:, ch * hw: (ch + 1) * hw]), MUL)
            nc.vector.tensor_tensor(
                _mb(st_sl(lh01, LB, ch, 1, q0, hw)), oo,
                _mb(th[:, N + ch * hw: N + (ch + 1) * hw]), MUL)
        oe, oo = _ev_od(go[2])
        nc.vector.tensor_tensor(_mb(lh2p[0:44, q0: q0 + hw]), oe,
                                _mb(th2[64:108, 0:hw]), MUL)
        nc.vector.tensor_tensor(_mb(lh2p[64:108, q0: q0 + hw]), oo,
                                _mb(th2[64:108, hw:N]), MUL)

    # ================================================================ phase A
    # leaves + level-1, software-pipelined: L1 GEMM of block b-1 is emitted
    # after the leaf GEMMs of block b so the PE never waits on leaf DVE.
    n_blk = XCOLS // LB                       # 8 blocks
    pend = None                               # (lh01, lh2p, lc01, lc2p, blk)

    def l1_block(lh01, lh2p, lc01, lc2p, blk):
        rhs_k = [st_sl(lh01, LB, 0, 0, 0, NF), st_sl(lh01, LB, 1, 0, 0, NF),
                 lh2p[:, :],
                 st_sl(lh01, LB, 0, 1, 0, NF), st_sl(lh01, LB, 1, 1, 0, NF)]
        G = gpool.tile([128, 13 * NF], F16, tag="G", name="G1")
        rec_gemm(rhs_k, NF, G)
        CL = [st_sl(lc01, LB, 0, 0, 0, NF), st_sl(lc01, LB, 1, 0, 0, NF),
              lc2p[64:108, :]]
        CR = [st_sl(lc01, LB, 0, 1, 0, NF), st_sl(lc01, LB, 1, 1, 0, NF),
              lc2p[0:44, :]]
        rec_ew(G, NF, CL, CR, ST[1], blk * NF)

    for blk in range(n_blk):
        c0 = blk * LB
        x0 = xpool.tile([128, LB], F16, tag="x0", name="x0")
        x1 = xpool.tile([128, LB], F16, tag="x1", name="x1")
        x2 = x2_t[blk % 2]
        nc.sync.dma_start(x0[:], xt_d[0:128, c0: c0 + LB])
        nc.sync.dma_start(x1[:], xt_d[128:256, c0: c0 + LB])
        nc.sync.dma_start(x2[0:44, :], xt_d[256:300, c0: c0 + LB])
        if blk == 0:
            for k in range(5):
                nc.sync.dma_start(wrec_sb[k][:], wrec_d[k])
        xk = [x0, x1, x2]

        lh01 = lpool.tile([128, 2 * LB], F16, tag="lh01", name="lh01")
        lh2p = lh2p_t[blk % 2]
        lc01 = lpool.tile([128, 2 * LB], F16, tag="lc01", name="lc01")
        lc2p = lpool.tile([128, LB // 2], F16, tag="lc2p", name="lc2p")

        Gls = []
        for s in range(2):
            Gl = glpool.tile([128, 8 * NF], F16, tag="Gl", name="Gl")
            leaf_gemm(xk, s, Gl)
            Gls.append(Gl)
        if pend is not None:
            l1_block(*pend)
        for s in range(2):
            leaf_ew(Gls[s], s, lh01, lh2p, lc01, lc2p)
        pend = (lh01, lh2p, lc01, lc2p, blk)
    l1_block(*pend)

    # ================================================================ phase B
    for lvl in range(2, 10):
        R = R_LVL[lvl]
        Rp = R_LVL[lvl - 1]
        PB = min(NF, R)
        prev = ST[lvl - 1]
        for p0 in range(0, R, PB):
            rhs_k = [st_sl(prev["h01"], Rp, 0, 0, p0, PB),
                     st_sl(prev["h01"], Rp, 1, 0, p0, PB),
                     prev["h2p"][:, p0: p0 + PB],
                     st_sl(prev["h01"], Rp, 0, 1, p0, PB),
                     st_sl(prev["h01"], Rp, 1, 1, p0, PB)]
            G = gpool.tile([128, 13 * NF], F16, tag="G", name=f"G{lvl}")
            rec_gemm(rhs_k, PB, G[:, : 13 * PB])
            CL = [st_sl(prev["c01"], Rp, 0, 0, p0, PB),
                  st_sl(prev["c01"], Rp, 1, 0, p0, PB),
                  prev["c2p"][64:108, p0: p0 + PB]]
            CR = [st_sl(prev["c01"], Rp, 0, 1, p0, PB),
                  st_sl(prev["c01"], Rp, 1, 1, p0, PB),
                  prev["c2p"][0:44, p0: p0 + PB]]
            if lvl < 9:
                rec_ew(G[:, : 13 * PB], PB, CL, CR, ST[lvl], p0)
            else:
                oc01 = opool.tile([128, 2 * B_LOC], F16, name="oc01")
                oc2 = opool.tile([44, B_LOC], F16, name="oc2")
                oh01 = opool.tile([128, 2 * B_LOC], F16, name="oh01")
                oh2 = opool.tile([44, B_LOC], F16, name="oh2")
                rec_ew(G[:, : 13 * PB], PB, CL, CR,
                       ("root", oc01, oc2, oh01, oh2), p0)
                nc.sync.dma_start(out_d[0, 0:128, :], oc01[:, 0:B_LOC])
                nc.sync.dma_start(out_d[0, 128:256, :], oc01[:, B_LOC: 2 * B_LOC])
                nc.sync.dma_start(out_d[0, 256:300, :], oc2[:, :])
                nc.sync.dma_start(out_d[1, 0:128, :], oh01[:, 0:B_LOC])
                nc.sync.dma_start(out_d[1, 128:256, :], oh01[:, B_LOC: 2 * B_LOC])
                nc.sync.dma_start(out_d[1, 256:300, :], oh2[:, :])


# ---------------------------------------------------------------- runner
_CACHE = {}


def _get_program():
    if "nc" not in _CACHE:
        _CACHE["nc"] = build_program()
    return _CACHE["nc"]


def _host_inputs(inputs, Wfioux, b_fioux, Wiouh, Wfh):
    wleaf, wrec = _pack_weights(
        np.asarray(Wfioux, np.float32), np.asarray(b_fioux, np.float32),
        np.asarray(Wiouh, np.float32), np.asarray(Wfh, np.float32))
    cons = np.zeros((84, 2 * LB), np.float16)
    cons[0, :] = 1.0
    in_maps = []
    for core in range(N_CORES):
        x = np.asarray(inputs[core * B_LOC:(core + 1) * B_LOC, :N_LEAVES, :],
                       np.float32)
        xt = np.ascontiguousarray(
            x.transpose(2, 1, 0).reshape(MEM, XCOLS)).astype(np.float16)
        in_maps.append({"xt": xt, "wleaf": wleaf, "wrec": wrec, "cons": cons})
    return in_maps


def kernel(inputs, Wfioux, b_fioux, Wiouh, Wfh, left_idx, right_idx, leaf_mask,
           _trace=False, _trace_dir=None):
    inputs = np.asarray(inputs, np.float32)
    assert _check_topology(left_idx, right_idx, leaf_mask), \
        "tree topology does not match the expected complete binary tree"

    in_maps = _host_inputs(inputs, Wfioux, b_fioux, Wiouh, Wfh)
    nc = _get_program()
    res = run_bass_kernel_spmd(nc, in_maps, list(range(N_CORES)),
                               trace=_trace, tmpdir=_trace_dir)

    root_c = np.zeros((B, MEM), np.float32)
    root_h = np.zeros((B, MEM), np.float32)
    for core in range(N_CORES):
        out = np.asarray(res.results[core]["out"], np.float32)  # [2, 300, 16]
        root_c[core * B_LOC:(core + 1) * B_LOC] = out[0].T
        root_h[core * B_LOC:(core + 1) * B_LOC] = out[1].T
    _CACHE["last_results"] = res
    return root_c, root_h


# revision 11
# speedup vs baseline: 1.3880x; 1.1372x over previous
"""Trainium2 Bass kernel for a batch-of-trees BinaryTreeLSTM (fp16 rewrite).

Contract: kernel(**inputs) takes the FULL inputs (B=128 trees, 1023-node
complete binary tree, dim 300) and returns the FULL output (root_c, root_h),
each [128, 300] float32.

Strategy
--------
- Data-parallel over trees: 16 trees per NeuronCore x 8 cores, no collectives.
- fp16 everywhere: GEMM operands (weights, x, h), gates, c/h states.  fp32
  PSUM accumulate + fp32 ACT internals keep the root error ~1e-3 (emulated),
  well under the 2e-2 gate.  fp16 runs 1 cycle/row on the PE at ANY free size
  (fp32r pays 4x below 256), halves LDWEIGHTS, DMA and SBUF vs fp32r.
- M-repacked gate units: the matmul cost model is (#units x #K-chunks) x N,
  independent of unit row-width, so the 1500 recurrent gate rows (i,o,u,fL,fR
  x 300) are packed into 12 units of <=128 rows (vs 15 naive) and the 900
  leaf gate rows into 8 units (vs 9).  Tail rows of several gates share units.
- Bias enters via a ones-row at partition 44 of the packed K-chunk 2 (the
  chunk that carries child-h features 256:300 of left/right at partitions
  0:44 / 64:108), so ACT applies pure sigmoid/tanh and pairs of units merge
  into single wide ACT instructions.
- All state is SBUF-resident (fp16 makes it fit); no DRAM round-trips for
  levels 1..3 anymore.  States stored deinterleaved ([even nodes | odd]) per
  feature chunk: h01/c01 [128, 2R] (chunks 0,1), h2p/c2p [128, R/2] with the
  44-row chunk-2 packed at partitions 0:44 (even) / 64:108 (odd).
- Phase A software-pipelines leaf blocks against the previous block's level-1
  GEMM so the PE never waits on the leaf elementwise chain.
"""

import os
import sys

for _p in ("/opt/trn_rl_repo",):
    if os.path.isdir(_p) and _p not in sys.path:
        sys.path.insert(0, _p)

import numpy as np
from contextlib import ExitStack

import concourse.bass as bass
import concourse.tile as tile
from concourse import bacc, mybir
from concourse.bass_utils import run_bass_kernel_spmd

# ---------------------------------------------------------------- constants
N_CORES = 8
B = 128
B_LOC = B // N_CORES          # 16 trees per core
N_LEAVES = 512
MEM = 300
XCOLS = N_LEAVES * B_LOC      # 8192 leaf columns per core
LB = 1024                     # leaf-block columns (64 leaves)
NF = 512                      # max moving free dim
R_LVL = {l: XCOLS >> l for l in range(1, 10)}   # level l column count

F16 = mybir.dt.float16
F32 = mybir.dt.float32
AF = mybir.ActivationFunctionType
SIG = AF.Sigmoid
TANH = AF.Tanh
MUL = mybir.AluOpType.mult
ADD = mybir.AluOpType.add

# Leaf M-units (8 units over Wiou cols [i 0:300 | o 300:600 | u 600:900]):
# unit -> list of (dst_row0, dst_row1, src_col0)
# (SBUF compute APs must start at partition 0 or 64, so tail gates sit at
# those offsets; rows 44:64 / 108:128 of tail units carry zero weights.)
LEAF_SLOTS = {
    0: [(0, 128, 0)], 1: [(0, 128, 128)],
    2: [(0, 128, 300)], 3: [(0, 128, 428)],
    4: [(0, 128, 600)], 5: [(0, 128, 728)],
    6: [(0, 44, 256), (64, 108, 556)],   # i2 @0 | o2 @64
    7: [(0, 44, 856)],                   # u2 @0
}
# Recurrent M-units (13 units over Wcat cols
# [i 0:300 | o 300:600 | u 600:900 | fL 900:1200 | fR 1200:1500]):
REC_SLOTS = {
    0: [(0, 128, 0)], 1: [(0, 128, 128)],
    2: [(0, 128, 300)], 3: [(0, 128, 428)],
    4: [(0, 128, 900)], 5: [(0, 128, 1028)],     # fL
    6: [(0, 128, 1200)], 7: [(0, 128, 1328)],    # fR
    8: [(0, 128, 600)], 9: [(0, 128, 728)],      # u
    10: [(0, 44, 256), (64, 108, 556)],          # i2 @0 | o2 @64
    11: [(0, 44, 856), (64, 108, 1156)],         # u2 @0 | fL2 @64
    12: [(0, 44, 1456)],                         # fR2 @0
}


# ---------------------------------------------------------------- host packing
def _pack_weights(Wfioux, b_fioux, Wiouh, Wfh):
    f4 = np.float32
    Wiou = np.asarray(Wfioux[:, 300:1200], f4)            # [300, 900]
    bleaf = np.asarray(b_fioux[300:1200], f4)             # [900]
    wleaf = np.zeros((3, 128, 8 * 128), f4)
    kch_l = [(0, 128), (128, 256), (256, 300)]
    for kc, (ra, rb) in enumerate(kch_l):
        for m, slots in LEAF_SLOTS.items():
            for (r0, r1, c0) in slots:
                wleaf[kc, 0: rb - ra, 128 * m + r0: 128 * m + r1] = \
                    Wiou[ra:rb, c0: c0 + (r1 - r0)]
    # bias via ones-row at partition 44 of K-chunk 2
    for m, slots in LEAF_SLOTS.items():
        for (r0, r1, c0) in slots:
            wleaf[2, 44, 128 * m + r0: 128 * m + r1] = bleaf[c0: c0 + (r1 - r0)]

    Wcat = np.concatenate([Wiouh, Wfh], axis=1).astype(f4)  # [600, 1500]
    bf = np.asarray(b_fioux[0:300], f4)
    bias_cat = np.concatenate(
        [b_fioux[300:600], b_fioux[600:900], b_fioux[900:1200], bf, bf]
    ).astype(f4)
    wrec = np.zeros((5, 128, 13 * 128), f4)
    # K-chunks: 0: hL[0:128], 1: hL[128:256], 2: packed hL[256:300]@0:44 +
    # ones@44 + hR[256:300]@64:108, 3: hR[0:128], 4: hR[128:256]
    kch_r = [(0, 128, 0), (128, 256, 0), None, (300, 428, 0), (428, 556, 0)]
    for kc, span in enumerate(kch_r):
        if span is None:
            continue
        ra, rb, _ = span
        for m, slots in REC_SLOTS.items():
            for (r0, r1, c0) in slots:
                wrec[kc, 0: rb - ra, 128 * m + r0: 128 * m + r1] = \
                    Wcat[ra:rb, c0: c0 + (r1 - r0)]
    for m, slots in REC_SLOTS.items():
        for (r0, r1, c0) in slots:
            wrec[2, 0:44, 128 * m + r0: 128 * m + r1] = \
                Wcat[256:300, c0: c0 + (r1 - r0)]
            wrec[2, 44, 128 * m + r0: 128 * m + r1] = bias_cat[c0: c0 + (r1 - r0)]
            wrec[2, 64:108, 128 * m + r0: 128 * m + r1] = \
                Wcat[556:600, c0: c0 + (r1 - r0)]
    return wleaf.astype(np.float16), wrec.astype(np.float16)


def _check_topology(left_idx, right_idx, leaf_mask):
    li = np.asarray(left_idx); ri = np.asarray(right_idx)
    prev = np.arange(N_LEAVES); nid = N_LEAVES
    ok = bool((np.asarray(leaf_mask)[:N_LEAVES] == 1).all())
    ok &= bool((np.asarray(leaf_mask)[N_LEAVES:] == 0).all())
    while len(prev) > 1:
        cur = []
        for k in range(0, len(prev), 2):
            ok &= bool(li[nid] == prev[k]) and bool(ri[nid] == prev[k + 1])
            cur.append(nid); nid += 1
        prev = np.asarray(cur)
    return ok


# ---------------------------------------------------------------- bass program
def _ev_od(ap, b=B_LOC):
    """Block-dense [p, X] (node-major, X = m*2*b) -> (even, odd) [p, m, b]."""
    r = ap.rearrange("p (m two b) -> p m two b", two=2, b=b)
    return r[:, :, 0, :], r[:, :, 1, :]


def _mb(ap, b=B_LOC):
    return ap.rearrange("p (m b) -> p m b", b=b)


def build_program():
    nc = bacc.Bacc("TRN2", target_bir_lowering=False, debug=False)

    xt_d = nc.dram_tensor("xt", [MEM, XCOLS], F16, kind="ExternalInput").ap()
    wleaf_d = nc.dram_tensor("wleaf", [3, 128, 8 * 128], F16,
                             kind="ExternalInput").ap()
    wrec_d = nc.dram_tensor("wrec", [5, 128, 13 * 128], F16,
                            kind="ExternalInput").ap()
    cons_d = nc.dram_tensor("cons", [84, 2 * LB], F16, kind="ExternalInput").ap()
    out_d = nc.dram_tensor("out", [2, MEM, B_LOC], F16, kind="ExternalOutput").ap()

    with ExitStack() as ctx:
        tc = ctx.enter_context(tile.TileContext(nc))
        _build(ctx, tc, xt_d, wleaf_d, wrec_d, cons_d, out_d)

    nc.compile()
    return nc


def _build(ctx, tc, xt_d, wleaf_d, wrec_d, cons_d, out_d):
    nc = tc.nc

    wpool = ctx.enter_context(tc.tile_pool(name="wpool", bufs=1))
    state_pool = ctx.enter_context(tc.tile_pool(name="state", bufs=1))

    # ---- weights resident in SBUF (leaf weights first: needed immediately)
    wleaf_sb = []
    for k in range(3):
        t = wpool.tile([128, 8 * 128], F16, name=f"wleaf{k}")
        nc.sync.dma_start(t[:], wleaf_d[k])
        wleaf_sb.append(t)
    wrec_sb = [wpool.tile([128, 13 * 128], F16, name=f"wrec{k}") for k in range(5)]

    # ---- persistent SBUF state for levels 1..8
    ST = {}
    for lvl in range(1, 9):
        R = R_LVL[lvl]
        h01 = state_pool.tile([128, 2 * R], F16, name=f"h01_{lvl}")
        h2p = state_pool.tile([128, R // 2], F16, name=f"h2p_{lvl}")
        c01 = state_pool.tile([128, 2 * R], F16, name=f"c01_{lvl}")
        c2p = state_pool.tile([128, R // 2], F16, name=f"c2p_{lvl}")
        ST[lvl] = dict(h01=h01, h2p=h2p, c01=c01, c2p=c2p, R=R)

    # persistent double-buffered leaf tiles that carry constant rows
    x2_t = []
    lh2p_t = []
    for i in range(2):
        t = state_pool.tile([128, LB], F16, name=f"x2_{i}")
        x2_t.append(t)
        t = state_pool.tile([128, LB // 2], F16, name=f"lh2p_{i}")
        lh2p_t.append(t)

    def _pad_dmas():
        """Pad/ones constants for tiles not needed in the first block; emitted
        after the block-0 x DMAs so compute starts immediately."""
        nc.sync.dma_start(x2_t[1][44:128, :], cons_d[0:84, :LB])
        for i in range(2):
            t = lh2p_t[i]
            nc.sync.dma_start(t[44:64, :], cons_d[0:20, : LB // 2])
            nc.sync.dma_start(t[108:128, :], cons_d[1:21, : LB // 2])
        for lvl in range(1, 9):
            R = R_LVL[lvl]
            h2p = ST[lvl]["h2p"]
            nc.sync.dma_start(h2p[44:64, :], cons_d[0:20, : R // 2])
            nc.sync.dma_start(h2p[108:128, :], cons_d[1:21, : R // 2])

    # ---- pools
    xpool = ctx.enter_context(tc.tile_pool(name="xpool", bufs=2))
    glpool = ctx.enter_context(tc.tile_pool(name="gl", bufs=3))
    lpool = ctx.enter_context(tc.tile_pool(name="lpool", bufs=2))
    gpool = ctx.enter_context(tc.tile_pool(name="g", bufs=2))
    pspool = ctx.enter_context(tc.tile_pool(name="ps", bufs=4, space="PSUM"))
    tmp1 = ctx.enter_context(tc.tile_pool(name="tmp1", bufs=1))
    tmp2 = ctx.enter_context(tc.tile_pool(name="tmp2", bufs=2))
    tmp3 = ctx.enter_context(tc.tile_pool(name="tmp3", bufs=1))
    opool = ctx.enter_context(tc.tile_pool(name="o", bufs=1))

    # ================================================================ helpers
    def leaf_gemm(xk, s, Gl):
        """Leaf gates for sub-chunk s (512 cols): 4 psum pairs, 6 ACTs."""
        n0 = s * NF
        for pi in range(4):
            ua, ub = 2 * pi, 2 * pi + 1
            ps = pspool.tile([128, 2 * NF], F32, tag="ps", name=f"psl{pi}")
            for j, u in enumerate((ua, ub)):
                rows = (128, 128, 128, 128, 128, 128, 108, 44)[u]
                off = j * NF
                for kc in range(3):
                    nc.tensor.matmul(
                        ps[0:rows, off: off + NF],
                        wleaf_sb[kc][:, 128 * u: 128 * u + rows],
                        xk[kc][:, n0: n0 + NF],
                        start=(kc == 0), stop=(kc == 2))
            if pi < 3:
                func = SIG if pi < 2 else TANH
                nc.scalar.activation(Gl[:, 2 * pi * NF: (2 * pi + 2) * NF],
                                     ps[:, :], func)
            else:
                nc.scalar.activation(Gl[0:108, 6 * NF: 7 * NF],
                                     ps[0:108, 0:NF], SIG)
                nc.scalar.activation(Gl[0:44, 7 * NF: 8 * NF],
                                     ps[0:44, NF: NF + NF], TANH)

    def rec_gemm(rhs_k, PB, G):
        """Recurrent gates for one block of PB cols: 6 psum pairs + 1 single."""
        UROWS = (128,) * 10 + (108, 108, 44)
        for pi in range(7):
            units = (2 * pi, 2 * pi + 1) if pi < 6 else (12,)
            ps = pspool.tile([128, 2 * NF], F32, tag="ps", name=f"psr{pi}")
            for j, u in enumerate(units):
                rows = UROWS[u]
                off = j * PB
                for kc in range(5):
                    nc.tensor.matmul(
                        ps[0:rows, off: off + PB],
                        wrec_sb[kc][:, 128 * u: 128 * u + rows],
                        rhs_k[kc],
                        start=(kc == 0), stop=(kc == 4))
            if pi < 5:
                func = SIG if pi < 4 else TANH
                nc.scalar.activation(G[:, 2 * pi * PB: (2 * pi + 2) * PB],
                                     ps[:, 0: 2 * PB], func)
            elif pi == 5:
                # T10 = [i2@0 | o2@64] all sigmoid; T11 = [u2@0 | fL2@64]
                nc.scalar.activation(G[0:108, 10 * PB: 11 * PB],
                                     ps[0:108, 0:PB], SIG)
                nc.scalar.activation(G[0:44, 11 * PB: 12 * PB],
                                     ps[0:44, PB: 2 * PB], TANH)
                nc.scalar.activation(G[64:108, 11 * PB: 12 * PB],
                                     ps[64:108, PB: 2 * PB], SIG)
            else:
                nc.scalar.activation(G[0:44, 12 * PB: 13 * PB],
                                     ps[0:44, 0:PB], SIG)

    def st_sl(t, R, ch, eo, q0, w):
        off = ch * R + eo * (R // 2) + q0
        return t[:, off: off + w]

    def rec_ew(G, PB, CL, CR, dst, p0):
        """Elementwise for a recurrent block. G gates [128, 12*PB].
        CL/CR: (c0, c1, c2) child-c dense APs [.,PB] (c2: 44 rows).
        dst: ST[lvl] dict, or ('root', oc01, oc2, oh01, oh2) for level 9."""
        N = PB
        gi = [G[:, 0:N], G[:, N: 2 * N], G[0:44, 10 * N: 11 * N]]
        go = [G[:, 2 * N: 3 * N], G[:, 3 * N: 4 * N], G[64:108, 10 * N: 11 * N]]
        gfL = [G[:, 4 * N: 5 * N], G[:, 5 * N: 6 * N], G[64:108, 11 * N: 12 * N]]
        gfR = [G[:, 6 * N: 7 * N], G[:, 7 * N: 8 * N], G[0:44, 12 * N: 13 * N]]
        gu = [G[:, 8 * N: 9 * N], G[:, 9 * N: 10 * N], G[0:44, 11 * N: 12 * N]]

        t1 = tmp1.tile([128, 2 * NF], F16, tag="t1", name="t1")
        t2 = tmp1.tile([128, 2 * NF], F16, tag="t2", name="t2")
        fc = tmp1.tile([128, 2 * NF], F16, tag="fc", name="fc")
        iu = tmp1.tile([128, 2 * NF], F16, tag="iu", name="iu")
        t1_2 = tmp1.tile([44, NF], F16, tag="t1_2", name="t1_2")
        t2_2 = tmp1.tile([44, NF], F16, tag="t2_2", name="t2_2")
        fc2 = tmp1.tile([44, NF], F16, tag="fc2", name="fc2")
        iu2 = tmp1.tile([44, NF], F16, tag="iu2", name="iu2")

        for ch in range(2):
            nc.vector.tensor_tensor(t1[:, ch * N: (ch + 1) * N], gfL[ch],
                                    CL[ch], MUL)
            nc.vector.tensor_tensor(t2[:, ch * N: (ch + 1) * N], gfR[ch],
                                    CR[ch], MUL)
        nc.vector.tensor_tensor(t1_2[:, :N], gfL[2], CL[2], MUL)
        nc.vector.tensor_tensor(t2_2[:, :N], gfR[2], CR[2], MUL)
        nc.vector.tensor_tensor(fc[:, : 2 * N], t1[:, : 2 * N], t2[:, : 2 * N],
                                ADD)
        nc.vector.tensor_tensor(fc2[:, :N], t1_2[:, :N], t2_2[:, :N], ADD)
        nc.vector.tensor_tensor(iu[:, :N], gi[0], gu[0], MUL)
        nc.vector.tensor_tensor(iu[:, N: 2 * N], gi[1], gu[1], MUL)
        nc.vector.tensor_tensor(iu2[:, :N], gi[2], gu[2], MUL)

        if isinstance(dst, tuple) and dst[0] == "root":
            _, oc01, oc2, oh01, oh2 = dst
            nc.vector.tensor_tensor(oc01[:, : 2 * N], iu[:, : 2 * N],
                                    fc[:, : 2 * N], ADD)
            nc.vector.tensor_tensor(oc2[:, :N], iu2[:, :N], fc2[:, :N], ADD)
            th = tmp2.tile([128, 2 * NF], F16, tag="th", name="th")
            th2 = tmp3.tile([128, NF], F16, tag="th2", name="th2")
            nc.scalar.activation(th[:, : 2 * N], oc01[:, : 2 * N], TANH)
            nc.scalar.activation(th2[64:108, :N], oc2[:, :N], TANH)
            nc.vector.tensor_tensor(oh01[:, :N], go[0], th[:, :N], MUL)
            nc.vector.tensor_tensor(oh01[:, N: 2 * N], go[1], th[:, N: 2 * N],
                                    MUL)
            nc.vector.tensor_tensor(oh2[:, :N], go[2], th2[64:108, :N], MUL)
            return

        st = dst
        R = st["R"]
        q0, hw = p0 // 2, PB // 2
        # c writes (deinterleave into state), then tanh, then h writes
        for ch in range(2):
            iue, iuo = _ev_od(iu[:, ch * N: (ch + 1) * N])
            fce, fco = _ev_od(fc[:, ch * N: (ch + 1) * N])
            nc.vector.tensor_tensor(_mb(st_sl(st["c01"], R, ch, 0, q0, hw)),
                                    iue, fce, ADD)
            nc.vector.tensor_tensor(_mb(st_sl(st["c01"], R, ch, 1, q0, hw)),
                                    iuo, fco, ADD)
        iue, iuo = _ev_od(iu2[:, :N])
        fce, fco = _ev_od(fc2[:, :N])
        nc.vector.tensor_tensor(_mb(st["c2p"][64:108, q0: q0 + hw]), iue, fce,
                                ADD)
        nc.vector.tensor_tensor(_mb(st["c2p"][0:44, q0: q0 + hw]), iuo, fco,
                                ADD)

        # th layout: [ch0E | ch1E | ch0O | ch1O], each hw wide
        th = tmp2.tile([128, 2 * NF], F16, tag="th", name="th")
        th2 = tmp3.tile([128, NF], F16, tag="th2", name="th2")
        c4 = st["c01"].rearrange("p (ch eo q) -> p ch eo q", ch=2, eo=2)
        tho = th[:, : 2 * N].rearrange("p (eo ch q) -> p eo ch q", eo=2, ch=2)
        nc.scalar.activation(tho[:, 0], c4[:, :, 0, q0: q0 + hw], TANH)
        nc.scalar.activation(tho[:, 1], c4[:, :, 1, q0: q0 + hw], TANH)
        nc.scalar.activation(th2[64:108, 0:hw], st["c2p"][64:108, q0: q0 + hw],
                             TANH)
        nc.scalar.activation(th2[64:108, hw:N], st["c2p"][0:44, q0: q0 + hw],
                             TANH)

        for ch in range(2):
            oe, oo = _ev_od(go[ch])
            nc.vector.tensor_tensor(_mb(st_sl(st["h01"], R, ch, 0, q0, hw)),
                                    oe, _mb(th[:, ch * hw: (ch + 1) * hw]), MUL)
            nc.vector.tensor_tensor(
                _mb(st_sl(st["h01"], R, ch, 1, q0, hw)), oo,
                _mb(th[:, N + ch * hw: N + (ch + 1) * hw]), MUL)
        oe, oo = _ev_od(go[2])
        nc.vector.tensor_tensor(_mb(st["h2p"][0:44, q0: q0 + hw]), oe,
                                _mb(th2[64:108, 0:hw]), MUL)
        nc.vector.tensor_tensor(_mb(st["h2p"][64:108, q0: q0 + hw]), oo,
                                _mb(th2[64:108, hw:N]), MUL)

    def leaf_ew(Gl, s, lh01, lh2p, lc01, lc2p):
        """Leaf elementwise for sub-chunk s (512 cols): c = i*u, h = o*tanh(c).
        Writes deinterleaved into the LB-wide block-local leaf tiles."""
        N = NF
        q0, hw = s * (NF // 2), NF // 2
        gi = [Gl[:, 0:N], Gl[:, N: 2 * N], Gl[0:44, 6 * N: 7 * N]]
        go = [Gl[:, 2 * N: 3 * N], Gl[:, 3 * N: 4 * N], Gl[64:108, 6 * N: 7 * N]]
        gu = [Gl[:, 4 * N: 5 * N], Gl[:, 5 * N: 6 * N], Gl[0:44, 7 * N: 8 * N]]

        for ch in range(2):
            ie, io = _ev_od(gi[ch])
            ue, uo = _ev_od(gu[ch])
            nc.vector.tensor_tensor(_mb(st_sl(lc01, LB, ch, 0, q0, hw)), ie, ue,
                                    MUL)
            nc.vector.tensor_tensor(_mb(st_sl(lc01, LB, ch, 1, q0, hw)), io, uo,
                                    MUL)
        i2e, i2o = _ev_od(gi[2])
        u2e, u2o = _ev_od(gu[2])
        nc.vector.tensor_tensor(_mb(lc2p[64:108, q0: q0 + hw]), i2e, u2e, MUL)
        nc.vector.tensor_tensor(_mb(lc2p[0:44, q0: q0 + hw]), i2o, u2o, MUL)

        th = tmp2.tile([128, 2 * NF], F16, tag="thl", name="lth")
        th2 = tmp3.tile([128, NF], F16, tag="th2l", name="lth2")
        c4 = lc01.rearrange("p (ch eo q) -> p ch eo q", ch=2, eo=2)
        tho = th[:, : 2 * N].rearrange("p (eo ch q) -> p eo ch q", eo=2, ch=2)
        nc.scalar.activation(tho[:, 0], c4[:, :, 0, q0: q0 + hw], TANH)
        nc.scalar.activation(tho[:, 1], c4[:, :, 1, q0: q0 + hw], TANH)
        nc.scalar.activation(th2[64:108, 0:hw], lc2p[64:108, q0: q0 + hw],
                             TANH)
        nc.scalar.activation(th2[64:108, hw:N], lc2p[0:44, q0: q0 + hw], TANH)

        for ch in range(2):
            oe, oo = _ev_od(go[ch])
            nc.vector.tensor_tensor(_mb(st_sl(lh01, LB, ch, 0, q0, hw)), oe,
                                    _mb(th[:, ch * hw: (ch + 1) * hw]), MUL)
            nc.vector.tensor_tensor(
                _mb(st_sl(lh01, LB, ch, 1, q0, hw)), oo,
                _mb(th[:, N + ch * hw: N + (ch + 1) * hw]), MUL)
        oe, oo = _ev_od(go[2])
        nc.vector.tensor_tensor(_mb(lh2p[0:44, q0: q0 + hw]), oe,
                                _mb(th2[64:108, 0:hw]), MUL)
        nc.vector.tensor_tensor(_mb(lh2p[64:108, q0: q0 + hw]), oo,
                                _mb(th2[64:108, hw:N]), MUL)

    # ================================================================ phase A
    # leaves + level-1, software-pipelined: L1 GEMM of block b-1 is emitted
    # after the leaf GEMMs of block b so the PE never waits on leaf DVE.
    n_blk = XCOLS // LB                       # 8 blocks
    pend = None                               # (lh01, lh2p, lc01, lc2p, blk)

    def l1_block(lh01, lh2p, lc01, lc2p, blk):
        rhs_k = [st_sl(lh01, LB, 0, 0, 0, NF), st_sl(lh01, LB, 1, 0, 0, NF),
                 lh2p[:, :],
                 st_sl(lh01, LB, 0, 1, 0, NF), st_sl(lh01, LB, 1, 1, 0, NF)]
        G = gpool.tile([128, 13 * NF], F16, tag="G", name="G1")
        rec_gemm(rhs_k, NF, G)
        CL = [st_sl(lc01, LB, 0, 0, 0, NF), st_sl(lc01, LB, 1, 0, 0, NF),
              lc2p[64:108, :]]
        CR = [st_sl(lc01, LB, 0, 1, 0, NF), st_sl(lc01, LB, 1, 1, 0, NF),
              lc2p[0:44, :]]
        rec_ew(G, NF, CL, CR, ST[1], blk * NF)

    for blk in range(n_blk):
        c0 = blk * LB
        x0 = xpool.tile([128, LB], F16, tag="x0", name="x0")
        x1 = xpool.tile([128, LB], F16, tag="x1", name="x1")
        x2 = x2_t[blk % 2]
        nc.sync.dma_start(x0[:], xt_d[0:128, c0: c0 + LB])
        nc.sync.dma_start(x1[:], xt_d[128:256, c0: c0 + LB])
        nc.sync.dma_start(x2[0:44, :], xt_d[256:300, c0: c0 + LB])
        if blk == 0:
            nc.sync.dma_start(x2[44:128, :], cons_d[0:84, :LB])
            for k in range(5):
                nc.sync.dma_start(wrec_sb[k][:], wrec_d[k])
            _pad_dmas()
        xk = [x0, x1, x2]

        lh01 = lpool.tile([128, 2 * LB], F16, tag="lh01", name="lh01")
        lh2p = lh2p_t[blk % 2]
        lc01 = lpool.tile([128, 2 * LB], F16, tag="lc01", name="lc01")
        lc2p = lpool.tile([128, LB // 2], F16, tag="lc2p", name="lc2p")

        Gls = []
        for s in range(2):
            Gl = glpool.tile([128, 8 * NF], F16, tag="Gl", name="Gl")
            leaf_gemm(xk, s, Gl)
            Gls.append(Gl)
        if pend is not None:
            l1_block(*pend)
        for s in range(2):
            leaf_ew(Gls[s], s, lh01, lh2p, lc01, lc2p)
        pend = (lh01, lh2p, lc01, lc2p, blk)
    l1_block(*pend)

    # ================================================================ phase B
    for lvl in range(2, 10):
        R = R_LVL[lvl]
        Rp = R_LVL[lvl - 1]
        PB = min(NF, R)
        prev = ST[lvl - 1]
        for p0 in range(0, R, PB):
            rhs_k = [st_sl(prev["h01"], Rp, 0, 0, p0, PB),
                     st_sl(prev["h01"], Rp, 1, 0, p0, PB),
                     prev["h2p"][:, p0: p0 + PB],
                     st_sl(prev["h01"], Rp, 0, 1, p0, PB),
                     st_sl(prev["h01"], Rp, 1, 1, p0, PB)]
            G = gpool.tile([128, 13 * NF], F16, tag="G", name=f"G{lvl}")
            rec_gemm(rhs_k, PB, G[:, : 13 * PB])
            CL = [st_sl(prev["c01"], Rp, 0, 0, p0, PB),
                  st_sl(prev["c01"], Rp, 1, 0, p0, PB),
                  prev["c2p"][64:108, p0: p0 + PB]]
            CR = [st_sl(prev["c01"], Rp, 0, 1, p0, PB),
                  st_sl(prev["c01"], Rp, 1, 1, p0, PB),
                  prev["c2p"][0:44, p0: p0 + PB]]
            if lvl < 9:
                rec_ew(G[:, : 13 * PB], PB, CL, CR, ST[lvl], p0)
            else:
                oc01 = opool.tile([128, 2 * B_LOC], F16, name="oc01")
                oc2 = opool.tile([44, B_LOC], F16, name="oc2")
                oh01 = opool.tile([128, 2 * B_LOC], F16, name="oh01")
                oh2 = opool.tile([44, B_LOC], F16, name="oh2")
                rec_ew(G[:, : 13 * PB], PB, CL, CR,
                       ("root", oc01, oc2, oh01, oh2), p0)
                nc.sync.dma_start(out_d[0, 0:128, :], oc01[:, 0:B_LOC])
                nc.sync.dma_start(out_d[0, 128:256, :], oc01[:, B_LOC: 2 * B_LOC])
                nc.sync.dma_start(out_d[0, 256:300, :], oc2[:, :])
                nc.sync.dma_start(out_d[1, 0:128, :], oh01[:, 0:B_LOC])
                nc.sync.dma_start(out_d[1, 128:256, :], oh01[:, B_LOC: 2 * B_LOC])
                nc.sync.dma_start(out_d[1, 256:300, :], oh2[:, :])


# ---------------------------------------------------------------- runner
_CACHE = {}


def _get_program():
    if "nc" not in _CACHE:
        _CACHE["nc"] = build_program()
    return _CACHE["nc"]


def _host_inputs(inputs, Wfioux, b_fioux, Wiouh, Wfh):
    wleaf, wrec = _pack_weights(
        np.asarray(Wfioux, np.float32), np.asarray(b_fioux, np.float32),
        np.asarray(Wiouh, np.float32), np.asarray(Wfh, np.float32))
    cons = np.zeros((84, 2 * LB), np.float16)
    cons[0, :] = 1.0
    in_maps = []
    for core in range(N_CORES):
        x = np.asarray(inputs[core * B_LOC:(core + 1) * B_LOC, :N_LEAVES, :],
                       np.float32)
        xt = np.ascontiguousarray(
            x.transpose(2, 1, 0).reshape(MEM, XCOLS)).astype(np.float16)
        in_maps.append({"xt": xt, "wleaf": wleaf, "wrec": wrec, "cons": cons})
    return in_maps


def kernel(inputs, Wfioux, b_fioux, Wiouh, Wfh, left_idx, right_idx, leaf_mask,
           _trace=False, _trace_dir=None):
    inputs = np.asarray(inputs, np.float32)
    assert _check_topology(left_idx, right_idx, leaf_mask), \
        "tree topology does not match the expected complete binary tree"

    in_maps = _host_inputs(inputs, Wfioux, b_fioux, Wiouh, Wfh)
    nc = _get_program()
    res = run_bass_kernel_spmd(nc, in_maps, list(range(N_CORES)),
                               trace=_trace, tmpdir=_trace_dir)

    root_c = np.zeros((B, MEM), np.float32)
    root_h = np.zeros((B, MEM), np.float32)
    for core in range(N_CORES):
        out = np.asarray(res.results[core]["out"], np.float32)  # [2, 300, 16]
        root_c[core * B_LOC:(core + 1) * B_LOC] = out[0].T
        root_h[core * B_LOC:(core + 1) * B_LOC] = out[1].T
    _CACHE["last_results"] = res
    return root_c, root_h
